# revision 1
# baseline (speedup 1.0000x reference)
"""Trainium2 Bass kernel for ConvBnSign (binarized 3x3 conv + sync-BN + sign).

Math: y = conv2d(x, sign(w) * alpha)  with alpha = mean|w| per out-channel,
then train-mode BatchNorm over (N,H,W), then hard_sign.

Since alpha_o > 0 is a per-channel scale, fold it into the BN affine:
  z = conv2d(x, sign(w))          (exact +-1 weights -> exact in bf16)
  y = alpha * z; mean_y = alpha*mu_z; var_y = alpha^2*var_z
  out = sign((z - mu_z) * A + beta)  with A = alpha*gamma*rsqrt(alpha^2 var_z + eps)
      = sign(z*A + B),  B = beta - mu_z*A

Precision: x is split on host into bf16 hi + lo (combined ~2^-18 relative);
each 3x3 tap is two accumulating bf16 matmuls into fp32 PSUM.

Sharding: data-parallel, 4 images per core across 8 cores; BN stats are
per-channel partial sums [128,4] fp32 all-reduced across cores.
"""

import numpy as np
import ml_dtypes

import concourse.bass as bass
import concourse.mybir as mybir
import concourse.tile as tile
from concourse.vector_clock import ScopedClock
from concourse.bass_utils import run_bass_kernel_spmd

# ---- problem constants (hardcoded per contract) ----
N_CORES = 8
N_FULL = 32           # batch
CIN = 128             # input channels
COUT = 256            # output channels
H = W = 56
KH = KW = 3
BN_EPS = 1e-5

IMGS = N_FULL // N_CORES          # 4 images per core
WP = W + 2                        # 58 padded width
HP = H + 2
PADPIX = HP * WP                  # 3364
PIX = H * W                       # 3136
NCHUNK = COUT // 128              # 2 chunks of 128 output channels
RTR = 8                           # rows per matmul tile
RT = H // RTR                     # 7 row tiles per image
NTILE = RTR * W                   # 448 = matmul free dim (<=512, one PSUM bank)
NTOT = N_FULL * PIX               # 200704 elements per channel for BN stats

BF16 = mybir.dt.bfloat16
F32 = mybir.dt.float32

_MAX_DRAIN_WAITS = 1  # walrus CTRL instructions accept a single sync wait


def _split_multi_waits(nc, max_waits=1):
    """This walrus build rejects instructions with more than one sem wait.
    Hoist excess waits onto same-engine NoOps inserted immediately before the
    offending instruction (the engine blocks at the NoOp instead — identical
    ordering semantics)."""
    ctr = 0
    for bbw in nc.main_func.blocks:
        out = []
        changed = False
        for inst in bbw.instructions:
            si = inst.sync_info
            w = list(si.on_wait or []) if si else []
            if len(w) > max_waits:
                changed = True
                excess = w[: len(w) - max_waits]
                for i in range(0, len(excess), max_waits):
                    nop = mybir.InstNoOp(name=f"WFIX-{ctr}", ins=[], outs=[])
                    ctr += 1
                    nop.engine = inst.engine
                    nop.sync_info = mybir.SyncInfo(
                        on_wait=excess[i : i + max_waits], on_update=[]
                    )
                    out.append(nop)
                inst.sync_info = mybir.SyncInfo(
                    on_wait=w[len(w) - max_waits :],
                    on_update=list(si.on_update or []),
                )
            out.append(inst)
        if changed:
            bbw.instructions = out
    return ctr


class _SplitDrainTileContext(tile.TileContext):
    """TileContext whose final drain splits its sem waits across multiple
    sync-engine instructions (this walrus build caps CTRL waits at 1)."""

    def _drain_and_barrier(self, tick_clock, wait_clock):
        drain_inst = self.nc.sync.drain()
        wait_clock.add_sem_waits(
            drain_inst.ins, ScopedClock({None: tick_clock.global_clock})
        )
        si = drain_inst.ins.sync_info
        w = list(si.on_wait or [])
        if len(w) > _MAX_DRAIN_WAITS:
            drain_inst.ins.sync_info = mybir.SyncInfo(
                on_wait=w[:_MAX_DRAIN_WAITS], on_update=list(si.on_update or [])
            )
            for i in range(_MAX_DRAIN_WAITS, len(w), _MAX_DRAIN_WAITS):
                nop = self.nc.sync.nop(nofuse=True)
                nop.ins.sync_info = mybir.SyncInfo(
                    on_wait=w[i : i + _MAX_DRAIN_WAITS], on_update=[]
                )
        self.nc.all_engine_barrier()
        assert self.sems is not None
        popped = self.nc._tile_sem_poison_stack.pop()
        assert popped is self._sem_poison
        self.nc.clear_and_free_semaphores(list(self.sems.allocated().values()))
        self.nc.all_engine_barrier()


def build_bass(n_cores=N_CORES, collective=True):
    """Build the per-core Bass module (SPMD: same program on every core)."""
    nc = bass.Bass(num_devices=n_cores)

    xh_d = nc.dram_tensor("xh", [IMGS, CIN, PADPIX], BF16, kind="ExternalInput")
    xl_d = nc.dram_tensor("xl", [IMGS, CIN, PADPIX], BF16, kind="ExternalInput")
    ws_d = nc.dram_tensor("ws", [CIN, KH * KW * COUT], BF16, kind="ExternalInput")
    abg_d = nc.dram_tensor("abg", [128, 3 * NCHUNK], F32, kind="ExternalInput")
    out_d = nc.dram_tensor("out", [IMGS, NCHUNK, 128, PIX], BF16,
                           kind="ExternalOutput")

    with _SplitDrainTileContext(nc) as tc:
        with (
            tc.tile_pool(name="const", bufs=1) as constp,
            tc.tile_pool(name="xbuf", bufs=1) as xp,
            tc.tile_pool(name="zbuf", bufs=1) as zp,
            tc.tile_pool(name="stats", bufs=1) as sp,
            tc.tile_pool(name="sq", bufs=2) as sqp,
            tc.tile_pool(name="pz", bufs=8, space="PSUM") as pp,
            tc.tile_pool(name="dram", bufs=1, space="DRAM") as dp,
        ):
            # ---- constants ----
            w_sb = constp.tile([128, KH * KW * COUT], BF16, tag="wsgn")
            abg_sb = constp.tile([128, 3 * NCHUNK], F32, tag="abg")
            nc.sync.dma_start(w_sb[:], ws_d[:])
            nc.sync.dma_start(abg_sb[:], abg_d[:])
            w_v = w_sb[:].rearrange("p (k o) -> p k o", k=KH * KW)

            # ---- x tiles (per image, hi/lo) ----
            xt = {}
            for img in range(IMGS):
                for half, src in (("h", xh_d), ("l", xl_d)):
                    t = xp.tile([128, PADPIX], BF16, tag=f"x{half}{img}", name=f"x{half}{img}")
                    nc.sync.dma_start(t[:], src[img])
                    xt[(half, img)] = t

            # ---- z buffers + stats ----
            z = [zp.tile([128, IMGS * PIX], F32, tag=f"z{j}", name=f"z{j}")
                 for j in range(NCHUNK)]
            ssum = sp.tile([128, 64], F32, tag="ssum")
            ssq = sp.tile([128, 64], F32, tag="ssq")

            alpha = abg_sb[:, 0:NCHUNK]
            gamma = abg_sb[:, NCHUNK : 2 * NCHUNK]
            beta = abg_sb[:, 2 * NCHUNK : 3 * NCHUNK]
            inv_n = 1.0 / NTOT
            npart = IMGS * RT

            # hi taps first: PE can start before any x_lo has arrived
            taps = [(k, "h") for k in range(KH * KW)] + \
                   [(k, "l") for k in range(KH * KW)]

            # Per chunk: conv -> stats AllReduce -> sign+store. Chunk 0's
            # collective + BN tail overlaps chunk 1's conv on PE.
            for j in range(NCHUNK):
                for img in range(IMGS):
                    # tile-major: one PSUM tile completes every 18 matmuls, so
                    # bank releases interleave smoothly with the next tile's
                    # compute (ldw-opt is off, so weight-major saved nothing)
                    for rt in range(RT):
                        pt = pp.tile([128, NTILE], F32, tag="pz",
                                     name=f"pz{j}_{img}_{rt}")
                        for widx, (k, half) in enumerate(taps):
                            dy, dx = divmod(k, KW)
                            lhsT = w_v[:, k, j * 128 : (j + 1) * 128]
                            xv = xt[(half, img)][:].rearrange(
                                "p (r c) -> p r c", r=HP
                            )
                            rhs = xv[:, rt * RTR + dy : rt * RTR + dy + RTR,
                                     dx : dx + W]
                            nc.tensor.matmul(
                                pt[:], lhsT, rhs,
                                start=(widx == 0), stop=(widx == len(taps) - 1),
                            )
                        col = img * RT + rt
                        zs = z[j][:, img * PIX + rt * NTILE
                                  : img * PIX + (rt + 1) * NTILE]
                        nc.vector.tensor_scalar(
                            out=zs, in0=pt[:], scalar1=0.0, scalar2=None,
                            op0=mybir.AluOpType.add, op1=mybir.AluOpType.add,
                            accum_out=ssum[:, j * npart + col
                                           : j * npart + col + 1],
                        )
                        sqt = sqp.tile([128, NTILE], F32, tag="sqt")
                        nc.scalar.activation(
                            out=sqt[:], in_=pt[:],
                            func=mybir.ActivationFunctionType.Square,
                            accum_out=ssq[:, j * npart + col
                                          : j * npart + col + 1],
                        )

                # ---- chunk-j stats: [128,2] = (sum, sumsq) ----
                cc_sb = sp.tile([128, 2], F32, tag=f"ccsb{j}", name=f"ccsb{j}")
                nc.vector.reduce_sum(
                    out=cc_sb[:, 0:1], in_=ssum[:, j * npart : (j + 1) * npart],
                    axis=mybir.AxisListType.X,
                )
                nc.vector.reduce_sum(
                    out=cc_sb[:, 1:2], in_=ssq[:, j * npart : (j + 1) * npart],
                    axis=mybir.AxisListType.X,
                )
                st = sp.tile([128, 2], F32, tag=f"st{j}", name=f"st{j}")
                if collective and n_cores > 1:
                    cc_in = dp.tile([128, 2], F32, tag=f"ccin{j}",
                                    name=f"ccin{j}")
                    cc_out = dp.tile([128, 2], F32, tag=f"ccout{j}",
                                     name=f"ccout{j}")
                    nc.sync.dma_start(cc_in[:], cc_sb[:])
                    nc.gpsimd.collective_compute(
                        "AllReduce", mybir.AluOpType.add,
                        replica_groups=[list(range(n_cores))],
                        ins=[cc_in.opt()], outs=[cc_out.opt()],
                    )
                    nc.sync.dma_start(st[:], cc_out[:])
                else:
                    nc.vector.tensor_copy(st[:], cc_sb[:])

                # ---- A, B for chunk j:  out = sign(z*A + B) ----
                al, ga, be = (v[:, j : j + 1] for v in (alpha, gamma, beta))
                mu = sp.tile([128, 1], F32, tag=f"mu{j}", name=f"mu{j}")
                var = sp.tile([128, 1], F32, tag=f"var{j}", name=f"var{j}")
                A = sp.tile([128, 1], F32, tag=f"A{j}", name=f"A{j}")
                B = sp.tile([128, 1], F32, tag=f"B{j}", name=f"B{j}")
                tmp = sp.tile([128, 1], F32, tag=f"tmp{j}", name=f"tmp{j}")

                nc.scalar.mul(mu[:], st[:, 0:1], inv_n)          # mu = s/n
                nc.scalar.mul(var[:], st[:, 1:2], inv_n)         # E[z^2]
                nc.vector.tensor_tensor(out=tmp[:], in0=mu[:], in1=mu[:],
                                        op=mybir.AluOpType.mult)
                nc.vector.tensor_tensor(out=var[:], in0=var[:], in1=tmp[:],
                                        op=mybir.AluOpType.subtract)
                nc.vector.tensor_tensor(out=tmp[:], in0=al, in1=al,
                                        op=mybir.AluOpType.mult)
                nc.vector.tensor_tensor(out=var[:], in0=var[:], in1=tmp[:],
                                        op=mybir.AluOpType.mult)
                nc.vector.tensor_scalar(out=var[:], in0=var[:],
                                        scalar1=float(BN_EPS), scalar2=None,
                                        op0=mybir.AluOpType.add)
                nc.scalar.sqrt(var[:], var[:])
                nc.vector.reciprocal(var[:], var[:])     # rsqrt(a^2 var + eps)
                nc.vector.tensor_tensor(out=tmp[:], in0=al, in1=ga,
                                        op=mybir.AluOpType.mult)
                nc.vector.tensor_tensor(out=A[:], in0=tmp[:], in1=var[:],
                                        op=mybir.AluOpType.mult)
                nc.vector.tensor_tensor(out=tmp[:], in0=mu[:], in1=A[:],
                                        op=mybir.AluOpType.mult)
                nc.vector.tensor_tensor(out=B[:], in0=be, in1=tmp[:],
                                        op=mybir.AluOpType.subtract)

                # ---- sign(z*A + B) -> bf16 staging -> DRAM ----
                for img in range(IMGS):
                    ostg = sqp.tile([128, PIX], BF16, tag="ostg",
                                    name=f"ostg{j}_{img}")
                    nc.scalar.activation(
                        out=ostg[:], in_=z[j][:, img * PIX : (img + 1) * PIX],
                        func=mybir.ActivationFunctionType.Sign,
                        bias=B[:, 0:1], scale=A[:, 0:1],
                    )
                    nc.sync.dma_start(out_d[img, j], ostg[:])

    _split_multi_waits(nc)
    return nc


def _prep_inputs(x, weight, gamma, beta):
    """Host-side prep: alpha/sign folding, padding, bf16 hi/lo split."""
    x = np.ascontiguousarray(x, dtype=np.float32)
    weight = np.ascontiguousarray(weight, dtype=np.float32)

    alpha = np.abs(weight).mean(axis=(1, 2, 3)).astype(np.float32)      # [256]
    sgn = np.where(weight >= 0, np.float32(1), np.float32(-1))          # [256,128,3,3]
    # ws[cin, k*256 + o] = sgn[o, cin, dy, dx],  k = dy*3+dx
    ws = np.ascontiguousarray(
        sgn.transpose(1, 2, 3, 0).reshape(CIN, KH * KW * COUT)
    ).astype(ml_dtypes.bfloat16)

    # abg[p, j] layout: [alpha(2) | gamma(2) | beta(2)], channel o = j*128+p
    def chunked(v):
        return np.ascontiguousarray(v.reshape(NCHUNK, 128).T)  # [128, 2]
    abg = np.concatenate(
        [chunked(alpha), chunked(np.asarray(gamma, np.float32)),
         chunked(np.asarray(beta, np.float32))], axis=1
    ).astype(np.float32)                                                # [128, 6]

    xpad = np.zeros((N_FULL, CIN, HP, WP), np.float32)
    xpad[:, :, 1 : H + 1, 1 : W + 1] = x
    xh = xpad.astype(ml_dtypes.bfloat16)
    xl = (xpad - xh.astype(np.float32)).astype(ml_dtypes.bfloat16)
    xh = xh.reshape(N_FULL, CIN, PADPIX)
    xl = xl.reshape(N_FULL, CIN, PADPIX)

    in_maps = []
    for c in range(N_CORES):
        sl = slice(c * IMGS, (c + 1) * IMGS)
        in_maps.append({
            "xh": np.ascontiguousarray(xh[sl]),
            "xl": np.ascontiguousarray(xl[sl]),
            "ws": ws,
            "abg": abg,
        })
    return in_maps


def kernel(x, weight, gamma, beta):
    in_maps = _prep_inputs(x, weight, gamma, beta)
    nc = build_bass()
    res = run_bass_kernel_spmd(nc, in_maps, core_ids=list(range(N_CORES)))
    out = np.empty((N_FULL, COUT, H, W), np.float32)
    for c in range(N_CORES):
        o = res.results[c]["out"]          # [IMGS, 2, 128, 3136] bf16 (+-1)
        o = o.astype(np.float32).reshape(IMGS, COUT, H, W)
        out[c * IMGS : (c + 1) * IMGS] = o
    return out



# revision 2
# speedup vs baseline: 1.9351x; 1.9351x over previous
"""Trainium2 Bass kernel for ConvBnSign (binarized 3x3 conv + sync-BN + sign).

Math: y = conv2d(x, sign(w) * alpha)  with alpha = mean|w| per out-channel,
then train-mode BatchNorm over (N,H,W), then hard_sign.

Since alpha_o > 0 is a per-channel scale, fold it into the BN affine:
  z = conv2d(x, sign(w))
  out = sign(z*A + B),  A = alpha*gamma*rsqrt(alpha^2 var_z + eps),
                        B = beta - mu_z*A

Precision: x is split on host into 3 fp8-e4m3 terms
  x ~ q0 + q1/16 + q2/64   (residual rms ~1.6e-5 relative),
and the per-term scales are folded into the fp8 sign-weights
(1, 2^-4, 2^-6 — all normal in e4m3). Each conv tile is then 14
DoubleRow fp8 matmuls (2 k-tiles of 128 each, 0.5 cyc/row) instead of
18 bf16 matmuls: the 27 (tap, term) k-tiles pair up via the plane dim
of a [5, PADPIX] SBUF layout, where planes 3/4 are host-shifted copies
of the q2 plane (shift +1 / +59) so cross-tap pairs land at a uniform
plane stride.

Sharding: data-parallel, 4 images per core across 8 cores; BN stats are
per-channel partial sums [128,4] fp32 all-reduced across cores.
"""

import numpy as np
import ml_dtypes

import concourse.bass as bass
import concourse.mybir as mybir
import concourse.tile as tile
from concourse.vector_clock import ScopedClock
from concourse.bass_utils import run_bass_kernel_spmd

# ---- problem constants (hardcoded per contract) ----
N_CORES = 8
N_FULL = 32           # batch
CIN = 128             # input channels
COUT = 256            # output channels
H = W = 56
KH = KW = 3
BN_EPS = 1e-5

IMGS = N_FULL // N_CORES          # 4 images per core
WP = W + 2                        # 58 padded width
HP = H + 2
PADPIX = HP * WP                  # 3364
PIX = H * W                       # 3136
NCHUNK = COUT // 128              # 2 chunks of 128 output channels
RTR = 8                           # rows per matmul tile
RT = H // RTR                     # 7 row tiles per image
NTILE = RTR * W                   # 448 = matmul free dim (<=512, one PSUM bank)
NTOT = N_FULL * PIX               # 200704 elements per channel for BN stats
NPLANE = 5                        # q0, q1, q2, q2<<1, q2<<59
NPAIR = 14                        # DoubleRow matmuls per PSUM tile

BF16 = mybir.dt.bfloat16
F32 = mybir.dt.float32
FP8 = mybir.dt.float8e4
NP8 = ml_dtypes.float8_e4m3

_MAX_DRAIN_WAITS = 1  # walrus CTRL instructions accept a single sync wait


def _split_multi_waits(nc, max_waits=1):
    """This walrus build rejects instructions with more than one sem wait.
    Hoist excess waits onto same-engine NoOps inserted immediately before the
    offending instruction (the engine blocks at the NoOp instead — identical
    ordering semantics)."""
    ctr = 0
    for bbw in nc.main_func.blocks:
        out = []
        changed = False
        for inst in bbw.instructions:
            si = inst.sync_info
            w = list(si.on_wait or []) if si else []
            if len(w) > max_waits:
                changed = True
                excess = w[: len(w) - max_waits]
                for i in range(0, len(excess), max_waits):
                    nop = mybir.InstNoOp(name=f"WFIX-{ctr}", ins=[], outs=[])
                    ctr += 1
                    nop.engine = inst.engine
                    nop.sync_info = mybir.SyncInfo(
                        on_wait=excess[i : i + max_waits], on_update=[]
                    )
                    out.append(nop)
                inst.sync_info = mybir.SyncInfo(
                    on_wait=w[len(w) - max_waits :],
                    on_update=list(si.on_update or []),
                )
            out.append(inst)
        if changed:
            bbw.instructions = out
    return ctr


class _SplitDrainTileContext(tile.TileContext):
    """TileContext whose final drain splits its sem waits across multiple
    sync-engine instructions (this walrus build caps CTRL waits at 1)."""

    def _drain_and_barrier(self, tick_clock, wait_clock):
        drain_inst = self.nc.sync.drain()
        wait_clock.add_sem_waits(
            drain_inst.ins, ScopedClock({None: tick_clock.global_clock})
        )
        si = drain_inst.ins.sync_info
        w = list(si.on_wait or [])
        if len(w) > _MAX_DRAIN_WAITS:
            drain_inst.ins.sync_info = mybir.SyncInfo(
                on_wait=w[:_MAX_DRAIN_WAITS], on_update=list(si.on_update or [])
            )
            for i in range(_MAX_DRAIN_WAITS, len(w), _MAX_DRAIN_WAITS):
                nop = self.nc.sync.nop(nofuse=True)
                nop.ins.sync_info = mybir.SyncInfo(
                    on_wait=w[i : i + _MAX_DRAIN_WAITS], on_update=[]
                )
        self.nc.all_engine_barrier()
        assert self.sems is not None
        popped = self.nc._tile_sem_poison_stack.pop()
        assert popped is self._sem_poison
        self.nc.clear_and_free_semaphores(list(self.sems.allocated().values()))
        self.nc.all_engine_barrier()


def _pair_rhs(xv, rt, pr):
    """rhs AP [128, 2, 8, 56] for DoubleRow pair pr of row-tile rt.

    xv: [128, NPLANE, HP, WP] view of the per-image fp8 plane stack."""
    r0 = rt * RTR
    if pr < 9:                       # (tap pr, q0) + (tap pr, q1)
        dy, dx = divmod(pr, KW)
        return xv[:, 0:2, r0 + dy : r0 + dy + RTR, dx : dx + W]
    if pr < 12:                      # q2 taps (dy,0)+(dy,1) via shift-1 plane
        dy = pr - 9
        return xv[:, 2:4, r0 + dy : r0 + dy + RTR, 0:W]
    if pr == 12:                     # q2 taps (0,2)+(1,2) via shift-59 plane
        return xv[:, 3:5, r0 : r0 + RTR, 1 : 1 + W]
    # pr == 13: q2 tap (2,2), second half has zero weights
    return xv[:, 2:3, r0 + 2 : r0 + 2 + RTR, 2 : 2 + W].broadcast_to(
        [128, 2, RTR, W]
    )


def build_bass(n_cores=N_CORES, collective=True):
    """Build the per-core Bass module (SPMD: same program on every core)."""
    nc = bass.Bass(num_devices=n_cores)

    xq_d = nc.dram_tensor("xq", [IMGS, CIN, NPLANE * PADPIX], FP8,
                          kind="ExternalInput")
    wq_d = nc.dram_tensor("wq", [CIN, NCHUNK * NPAIR * 2 * 128], FP8,
                          kind="ExternalInput")
    abg_d = nc.dram_tensor("abg", [128, 3 * NCHUNK], F32, kind="ExternalInput")
    out_d = nc.dram_tensor("out", [IMGS, NCHUNK, 128, PIX], BF16,
                           kind="ExternalOutput")

    with _SplitDrainTileContext(nc) as tc:
        with (
            tc.tile_pool(name="const", bufs=1) as constp,
            tc.tile_pool(name="xbuf", bufs=1) as xp,
            tc.tile_pool(name="zbuf", bufs=1) as zp,
            tc.tile_pool(name="stats", bufs=1) as sp,
            tc.tile_pool(name="sq", bufs=2) as sqp,
            tc.tile_pool(name="pz", bufs=8, space="PSUM") as pp,
            tc.tile_pool(name="dram", bufs=1, space="DRAM") as dp,
        ):
            # ---- constants ----
            w_sb = constp.tile([128, NCHUNK * NPAIR * 2 * 128], FP8, tag="wq")
            abg_sb = constp.tile([128, 3 * NCHUNK], F32, tag="abg")
            nc.sync.dma_start(w_sb[:], wq_d[:])
            nc.sync.dma_start(abg_sb[:], abg_d[:])
            w_v = w_sb[:].rearrange("p (j pr k o) -> p j pr k o",
                                    j=NCHUNK, pr=NPAIR, k=2)

            # ---- x plane stacks (per image) ----
            xt = []
            for img in range(IMGS):
                t = xp.tile([128, NPLANE * PADPIX], FP8, tag=f"x{img}",
                            name=f"x{img}")
                nc.sync.dma_start(t[:], xq_d[img])
                xt.append(t)

            # ---- z buffers + stats ----
            z = [zp.tile([128, IMGS * PIX], F32, tag=f"z{j}", name=f"z{j}")
                 for j in range(NCHUNK)]
            ssum = sp.tile([128, 64], F32, tag="ssum")
            ssq = sp.tile([128, 64], F32, tag="ssq")

            alpha = abg_sb[:, 0:NCHUNK]
            gamma = abg_sb[:, NCHUNK : 2 * NCHUNK]
            beta = abg_sb[:, 2 * NCHUNK : 3 * NCHUNK]
            inv_n = 1.0 / NTOT
            npart = IMGS * RT

            # Per chunk: conv -> stats AllReduce -> sign+store. Chunk 0's
            # collective + BN tail overlaps chunk 1's conv on PE.
            for j in range(NCHUNK):
                for img in range(IMGS):
                    xv = xt[img][:].rearrange("p (t r c) -> p t r c",
                                              t=NPLANE, r=HP)
                    for rt in range(RT):
                        pt = pp.tile([128, NTILE], F32, tag="pz",
                                     name=f"pz{j}_{img}_{rt}")
                        for pr in range(NPAIR):
                            nc.tensor.matmul(
                                pt[:], w_v[:, j, pr], _pair_rhs(xv, rt, pr),
                                start=(pr == 0), stop=(pr == NPAIR - 1),
                                perf_mode=mybir.MatmulPerfMode.DoubleRow,
                            )
                        col = img * RT + rt
                        zs = z[j][:, img * PIX + rt * NTILE
                                  : img * PIX + (rt + 1) * NTILE]
                        nc.vector.tensor_scalar(
                            out=zs, in0=pt[:], scalar1=0.0, scalar2=None,
                            op0=mybir.AluOpType.add, op1=mybir.AluOpType.add,
                            accum_out=ssum[:, j * npart + col
                                           : j * npart + col + 1],
                        )
                        sqt = sqp.tile([128, NTILE], F32, tag="sqt")
                        nc.scalar.activation(
                            out=sqt[:], in_=pt[:],
                            func=mybir.ActivationFunctionType.Square,
                            accum_out=ssq[:, j * npart + col
                                          : j * npart + col + 1],
                        )

                # ---- chunk-j stats: [128,2] = (sum, sumsq) ----
                cc_sb = sp.tile([128, 2], F32, tag=f"ccsb{j}", name=f"ccsb{j}")
                nc.vector.reduce_sum(
                    out=cc_sb[:, 0:1], in_=ssum[:, j * npart : (j + 1) * npart],
                    axis=mybir.AxisListType.X,
                )
                nc.vector.reduce_sum(
                    out=cc_sb[:, 1:2], in_=ssq[:, j * npart : (j + 1) * npart],
                    axis=mybir.AxisListType.X,
                )
                st = sp.tile([128, 2], F32, tag=f"st{j}", name=f"st{j}")
                if collective and n_cores > 1:
                    cc_in = dp.tile([128, 2], F32, tag=f"ccin{j}",
                                    name=f"ccin{j}")
                    cc_out = dp.tile([128, 2], F32, tag=f"ccout{j}",
                                     name=f"ccout{j}")
                    nc.sync.dma_start(cc_in[:], cc_sb[:])
                    nc.gpsimd.collective_compute(
                        "AllReduce", mybir.AluOpType.add,
                        replica_groups=[list(range(n_cores))],
                        ins=[cc_in.opt()], outs=[cc_out.opt()],
                    )
                    nc.sync.dma_start(st[:], cc_out[:])
                else:
                    nc.vector.tensor_copy(st[:], cc_sb[:])

                # ---- A, B for chunk j:  out = sign(z*A + B) ----
                al, ga, be = (v[:, j : j + 1] for v in (alpha, gamma, beta))
                mu = sp.tile([128, 1], F32, tag=f"mu{j}", name=f"mu{j}")
                var = sp.tile([128, 1], F32, tag=f"var{j}", name=f"var{j}")
                A = sp.tile([128, 1], F32, tag=f"A{j}", name=f"A{j}")
                B = sp.tile([128, 1], F32, tag=f"B{j}", name=f"B{j}")
                tmp = sp.tile([128, 1], F32, tag=f"tmp{j}", name=f"tmp{j}")

                nc.scalar.mul(mu[:], st[:, 0:1], inv_n)          # mu = s/n
                nc.scalar.mul(var[:], st[:, 1:2], inv_n)         # E[z^2]
                nc.vector.tensor_tensor(out=tmp[:], in0=mu[:], in1=mu[:],
                                        op=mybir.AluOpType.mult)
                nc.vector.tensor_tensor(out=var[:], in0=var[:], in1=tmp[:],
                                        op=mybir.AluOpType.subtract)
                nc.vector.tensor_tensor(out=tmp[:], in0=al, in1=al,
                                        op=mybir.AluOpType.mult)
                nc.vector.tensor_tensor(out=var[:], in0=var[:], in1=tmp[:],
                                        op=mybir.AluOpType.mult)
                nc.vector.tensor_scalar(out=var[:], in0=var[:],
                                        scalar1=float(BN_EPS), scalar2=None,
                                        op0=mybir.AluOpType.add)
                nc.scalar.sqrt(var[:], var[:])
                nc.vector.reciprocal(var[:], var[:])     # rsqrt(a^2 var + eps)
                nc.vector.tensor_tensor(out=tmp[:], in0=al, in1=ga,
                                        op=mybir.AluOpType.mult)
                nc.vector.tensor_tensor(out=A[:], in0=tmp[:], in1=var[:],
                                        op=mybir.AluOpType.mult)
                nc.vector.tensor_tensor(out=tmp[:], in0=mu[:], in1=A[:],
                                        op=mybir.AluOpType.mult)
                nc.vector.tensor_tensor(out=B[:], in0=be, in1=tmp[:],
                                        op=mybir.AluOpType.subtract)

                # ---- sign(z*A + B) -> bf16 staging -> DRAM ----
                for img in range(IMGS):
                    ostg = sqp.tile([128, PIX], BF16, tag="ostg",
                                    name=f"ostg{j}_{img}")
                    nc.scalar.activation(
                        out=ostg[:], in_=z[j][:, img * PIX : (img + 1) * PIX],
                        func=mybir.ActivationFunctionType.Sign,
                        bias=B[:, 0:1], scale=A[:, 0:1],
                    )
                    nc.sync.dma_start(out_d[img, j], ostg[:])

    _split_multi_waits(nc)
    return nc


def _prep_inputs(x, weight, gamma, beta):
    """Host-side prep: alpha/sign folding, padding, 3-term fp8 split."""
    x = np.ascontiguousarray(x, dtype=np.float32)
    weight = np.ascontiguousarray(weight, dtype=np.float32)

    alpha = np.abs(weight).mean(axis=(1, 2, 3)).astype(np.float32)      # [256]
    sgn = np.where(weight >= 0, np.float32(1), np.float32(-1))          # [256,128,3,3]

    # ---- fp8 weight pairs: wq[cin, j, pair, ktile, o] ----
    # sgn_t[cin, k, j, o]: tap k = dy*3+dx, chunk j, cout o
    sgn_t = sgn.transpose(1, 2, 3, 0).reshape(CIN, KH * KW, NCHUNK, 128)
    wq = np.zeros((CIN, NCHUNK, NPAIR, 2, 128), np.float32)
    S1, S2 = 2.0 ** -4, 2.0 ** -6
    for k in range(9):                      # pairs 0-8: (q0, q1) of tap k
        wq[:, :, k, 0] = sgn_t[:, k]
        wq[:, :, k, 1] = sgn_t[:, k] * S1
    for dy in range(3):                     # pairs 9-11: q2 taps (dy,0)+(dy,1)
        wq[:, :, 9 + dy, 0] = sgn_t[:, dy * 3 + 0] * S2
        wq[:, :, 9 + dy, 1] = sgn_t[:, dy * 3 + 1] * S2
    wq[:, :, 12, 0] = sgn_t[:, 2] * S2      # pair 12: q2 taps (0,2)+(1,2)
    wq[:, :, 12, 1] = sgn_t[:, 5] * S2
    wq[:, :, 13, 0] = sgn_t[:, 8] * S2      # pair 13: q2 tap (2,2) + zeros
    wq = np.ascontiguousarray(
        wq.reshape(CIN, NCHUNK * NPAIR * 2 * 128)
    ).astype(NP8)

    # abg[p, j] layout: [alpha(2) | gamma(2) | beta(2)], channel o = j*128+p
    def chunked(v):
        return np.ascontiguousarray(v.reshape(NCHUNK, 128).T)  # [128, 2]
    abg = np.concatenate(
        [chunked(alpha), chunked(np.asarray(gamma, np.float32)),
         chunked(np.asarray(beta, np.float32))], axis=1
    ).astype(np.float32)                                                # [128, 6]

    # ---- 3-term fp8 split of padded x, with shifted q2 planes ----
    xpad = np.zeros((N_FULL, CIN, HP, WP), np.float32)
    xpad[:, :, 1 : H + 1, 1 : W + 1] = x
    xpad = xpad.reshape(N_FULL, CIN, PADPIX)
    q0 = xpad.astype(NP8)
    r1 = xpad - q0.astype(np.float32)
    q1 = (r1 * 16.0).astype(NP8)
    r2 = r1 - q1.astype(np.float32) * (1.0 / 16.0)
    q2 = (r2 * 64.0).astype(NP8)
    q2p = np.zeros((N_FULL, CIN, PADPIX + 64), NP8)
    q2p[:, :, :PADPIX] = q2
    xq = np.stack(
        [q0, q1, q2, q2p[:, :, 1 : 1 + PADPIX], q2p[:, :, 59 : 59 + PADPIX]],
        axis=2,
    )                                                   # [N, CIN, 5, PADPIX]
    xq = np.ascontiguousarray(xq.reshape(N_FULL, CIN, NPLANE * PADPIX))

    in_maps = []
    for c in range(N_CORES):
        sl = slice(c * IMGS, (c + 1) * IMGS)
        in_maps.append({
            "xq": np.ascontiguousarray(xq[sl]),
            "wq": wq,
            "abg": abg,
        })
    return in_maps


def kernel(x, weight, gamma, beta):
    in_maps = _prep_inputs(x, weight, gamma, beta)
    nc = build_bass()
    res = run_bass_kernel_spmd(nc, in_maps, core_ids=list(range(N_CORES)))
    out = np.empty((N_FULL, COUT, H, W), np.float32)
    for c in range(N_CORES):
        o = res.results[c]["out"]          # [IMGS, 2, 128, 3136] bf16 (+-1)
        o = o.astype(np.float32).reshape(IMGS, COUT, H, W)
        out[c * IMGS : (c + 1) * IMGS] = o
    return out


# revision 9
# speedup vs baseline: 2.0242x; 1.0460x over previous
"""Trainium2 Bass kernel for ConvBnSign (binarized 3x3 conv + sync-BN + sign).

Math: y = conv2d(x, sign(w) * alpha)  with alpha = mean|w| per out-channel,
then train-mode BatchNorm over (N,H,W), then hard_sign.

Folds: alpha > 0 folds into the BN affine; S = sign(gamma) folds into the
binarized weights (z' = S*z), making the BN scale A = alpha*|gamma|*rsqrt(
alpha^2 var + eps) >= 0 so the final sign is also a per-channel threshold
compare  out = (z' >= T) ? +1 : -1,  T = mu' - beta/A  — which lets the
tail's sign pass split across ACT / DVE / Pool engines.

Precision: x is split on host into 3 fp8-e4m3 terms
  x ~ q0 + q1/16 + q2/64   (residual rms ~1.6e-5 relative),
with per-term scales folded into the fp8 weights (1, 2^-4, 2^-6 — all
normal in e4m3). Each conv tile is 14 DoubleRow fp8 matmuls (2 k-tiles
each, 0.5 cyc/row): the 27 (tap, term) k-tiles pair via the plane dim of
the SBUF layout, where two extra planes are host-shifted copies of q2
(shift +1 / +59) so cross-tap pairs land at the uniform plane stride.

Per-tile stats consumers both run on DVE (tensor_scalar z-copy+sum and
tensor_tensor_reduce square+sum), keeping ACT free for Sign and Pool free
for reduces/BN math — avoids ACT head-of-line blocking on PSUM drains.

Sharding: data-parallel, 4 images per core across 8 cores; BN stats are
per-channel partial sums [128,4] fp32 all-reduced across cores.
"""

import numpy as np
import ml_dtypes

import concourse.bass as bass
import concourse.mybir as mybir
import concourse.tile as tile
from concourse.vector_clock import ScopedClock
from concourse.bass_utils import run_bass_kernel_spmd

# ---- problem constants (hardcoded per contract) ----
N_CORES = 8
N_FULL = 32           # batch
CIN = 128             # input channels
COUT = 256            # output channels
H = W = 56
KH = KW = 3
BN_EPS = 1e-5

IMGS = N_FULL // N_CORES          # 4 images per core
WP = W + 2                        # 58 padded width
HP = H + 2
PADPIX = HP * WP                  # 3364
PIX = H * W                       # 3136
NCHUNK = COUT // 128              # 2 chunks of 128 output channels
RTR = 8                           # rows per matmul tile
RT = H // RTR                     # 7 row tiles per image
NTILE = RTR * W                   # 448 = matmul free dim (<=512, one PSUM bank)
NTOT = N_FULL * PIX               # 200704 elements per channel for BN stats
NPAIR = 14                        # DoubleRow matmuls per PSUM tile
WSLOT = 2 * 128                   # fp8 bytes per weight pair
PHA = 9 * WSLOT                   # phase-1 weight bytes (pairs 0-8)
PHB = 5 * WSLOT                   # phase-2 weight bytes (pairs 9-13)

BF16 = mybir.dt.bfloat16
F32 = mybir.dt.float32
FP8 = mybir.dt.float8e4
NP8 = ml_dtypes.float8_e4m3

_MAX_DRAIN_WAITS = 1  # walrus CTRL instructions accept a single sync wait


def _split_multi_waits(nc, max_waits=1):
    """This walrus build rejects instructions with more than one sem wait.
    Hoist excess waits onto same-engine NoOps inserted immediately before the
    offending instruction (the engine blocks at the NoOp instead — identical
    ordering semantics)."""
    ctr = 0
    for bbw in nc.main_func.blocks:
        out = []
        changed = False
        for inst in bbw.instructions:
            si = inst.sync_info
            w = list(si.on_wait or []) if si else []
            if len(w) > max_waits:
                changed = True
                excess = w[: len(w) - max_waits]
                for i in range(0, len(excess), max_waits):
                    nop = mybir.InstNoOp(name=f"WFIX-{ctr}", ins=[], outs=[])
                    ctr += 1
                    nop.engine = inst.engine
                    nop.sync_info = mybir.SyncInfo(
                        on_wait=excess[i : i + max_waits], on_update=[]
                    )
                    out.append(nop)
                inst.sync_info = mybir.SyncInfo(
                    on_wait=w[len(w) - max_waits :],
                    on_update=list(si.on_update or []),
                )
            out.append(inst)
        if changed:
            bbw.instructions = out
    return ctr


class _SplitDrainTileContext(tile.TileContext):
    """TileContext whose final drain splits its sem waits across multiple
    sync-engine instructions (this walrus build caps CTRL waits at 1)."""

    def _drain_and_barrier(self, tick_clock, wait_clock):
        drain_inst = self.nc.sync.drain()
        wait_clock.add_sem_waits(
            drain_inst.ins, ScopedClock({None: tick_clock.global_clock})
        )
        si = drain_inst.ins.sync_info
        w = list(si.on_wait or [])
        if len(w) > _MAX_DRAIN_WAITS:
            drain_inst.ins.sync_info = mybir.SyncInfo(
                on_wait=w[:_MAX_DRAIN_WAITS], on_update=list(si.on_update or [])
            )
            for i in range(_MAX_DRAIN_WAITS, len(w), _MAX_DRAIN_WAITS):
                nop = self.nc.sync.nop(nofuse=True)
                nop.ins.sync_info = mybir.SyncInfo(
                    on_wait=w[i : i + _MAX_DRAIN_WAITS], on_update=[]
                )
        self.nc.all_engine_barrier()
        assert self.sems is not None
        popped = self.nc._tile_sem_poison_stack.pop()
        assert popped is self._sem_poison
        self.nc.clear_and_free_semaphores(list(self.sems.allocated().values()))
        self.nc.all_engine_barrier()


def _pair_rhs(xa_v, xb_v, rt, pr):
    """rhs AP [128, 2, 8, 56] for DoubleRow pair pr of row-tile rt.

    xa_v: [128, 2, HP, WP] view of planes (q0, q1);
    xb_v: [128, 3, HP, WP] view of planes (q2, q2<<1, q2<<59)."""
    r0 = rt * RTR
    if pr < 9:                       # (tap pr, q0) + (tap pr, q1)
        dy, dx = divmod(pr, KW)
        return xa_v[:, 0:2, r0 + dy : r0 + dy + RTR, dx : dx + W]
    if pr < 12:                      # q2 taps (dy,0)+(dy,1) via shift-1 plane
        dy = pr - 9
        return xb_v[:, 0:2, r0 + dy : r0 + dy + RTR, 0:W]
    if pr == 12:                     # q2 taps (0,2)+(1,2) via shift-59 plane
        return xb_v[:, 1:3, r0 : r0 + RTR, 1 : 1 + W]
    # pr == 13: q2 tap (2,2), second half has zero weights
    return xb_v[:, 0:1, r0 + 2 : r0 + 2 + RTR, 2 : 2 + W].broadcast_to(
        [128, 2, RTR, W]
    )


def build_bass(n_cores=N_CORES, collective=True):
    """Build the per-core Bass module (SPMD: same program on every core)."""
    nc = bass.Bass(num_devices=n_cores)

    xq_d = nc.dram_tensor("xq", [IMGS, CIN, 5 * PADPIX], FP8,
                          kind="ExternalInput")
    wq_d = nc.dram_tensor("wq", [CIN, NCHUNK * NPAIR * WSLOT], FP8,
                          kind="ExternalInput")
    # pqrb[p, j-col chunks]: P | Qc | R | beta  (4 cols per chunk)
    pqrb_d = nc.dram_tensor("pqrb", [128, 4 * NCHUNK], F32,
                            kind="ExternalInput")
    out_d = nc.dram_tensor("out", [IMGS, NCHUNK, 128, PIX], BF16,
                           kind="ExternalOutput")

    with _SplitDrainTileContext(nc) as tc:
        with (
            tc.tile_pool(name="const", bufs=1) as constp,
            tc.tile_pool(name="xbuf", bufs=1) as xp,
            tc.tile_pool(name="zbuf", bufs=1) as zp,
            tc.tile_pool(name="stats", bufs=1) as sp,
            tc.tile_pool(name="sq", bufs=2) as sqp,
            tc.tile_pool(name="ost", bufs=2) as op_,
            tc.tile_pool(name="pz", bufs=8, space="PSUM") as pp,
            tc.tile_pool(name="dram", bufs=1, space="DRAM") as dp,
        ):
            # ---- weights (split per chunk/phase for early PE start) ----
            wa = [constp.tile([128, PHA], FP8, tag=f"wa{j}", name=f"wa{j}")
                  for j in range(NCHUNK)]
            wb = [constp.tile([128, PHB], FP8, tag=f"wb{j}", name=f"wb{j}")
                  for j in range(NCHUNK)]
            pqrb_sb = constp.tile([128, 4 * NCHUNK], F32, tag="pqrb")

            # ---- x plane stacks (per image, split planes 01 / 234) ----
            xa = [xp.tile([128, 2 * PADPIX], FP8, tag=f"xa{i}", name=f"xa{i}")
                  for i in range(IMGS)]
            xb = [xp.tile([128, 3 * PADPIX], FP8, tag=f"xb{i}", name=f"xb{i}")
                  for i in range(IMGS)]

            # DMA issue order = arrival order: w0a, img0 planes, w0b, then
            # the rest.  First matmul waits only on w0a + xa[0].
            nc.sync.dma_start(wa[0][:], wq_d[:, 0:PHA])
            nc.sync.dma_start(xa[0][:], xq_d[0][:, 0 : 2 * PADPIX])
            nc.sync.dma_start(wb[0][:], wq_d[:, PHA : PHA + PHB])
            nc.sync.dma_start(xb[0][:], xq_d[0][:, 2 * PADPIX :])
            nc.sync.dma_start(pqrb_sb[:], pqrb_d[:])
            for i in range(1, IMGS):
                nc.sync.dma_start(xa[i][:], xq_d[i][:, 0 : 2 * PADPIX])
                nc.sync.dma_start(xb[i][:], xq_d[i][:, 2 * PADPIX :])
            off = NPAIR * WSLOT
            nc.sync.dma_start(wa[1][:], wq_d[:, off : off + PHA])
            nc.sync.dma_start(wb[1][:], wq_d[:, off + PHA : off + PHA + PHB])

            # ---- z buffers + stats ----
            z = [zp.tile([128, IMGS * PIX], F32, tag=f"z{j}", name=f"z{j}")
                 for j in range(NCHUNK)]
            ssum = sp.tile([128, 64], F32, tag="ssum")
            ssq = sp.tile([128, 64], F32, tag="ssq")

            P_ = pqrb_sb[:, 0:NCHUNK]
            Qc = pqrb_sb[:, NCHUNK : 2 * NCHUNK]
            R_ = pqrb_sb[:, 2 * NCHUNK : 3 * NCHUNK]
            beta = pqrb_sb[:, 3 * NCHUNK : 4 * NCHUNK]
            inv_n = 1.0 / NTOT
            npart = IMGS * RT

            for j in range(NCHUNK):
                wa_v = wa[j][:].rearrange("p (pr k o) -> p pr k o", pr=9, k=2)
                wb_v = wb[j][:].rearrange("p (pr k o) -> p pr k o", pr=5, k=2)

                for img in range(IMGS):
                    xa_v = xa[img][:].rearrange("p (t r c) -> p t r c",
                                                t=2, r=HP)
                    xb_v = xb[img][:].rearrange("p (t r c) -> p t r c",
                                                t=3, r=HP)
                    pts = [pp.tile([128, NTILE], F32, tag="pz",
                                   name=f"pz{j}_{img}_{rt}")
                           for rt in range(RT)]
                    # phase 1: pairs 0-8 (planes q0/q1) across all row tiles
                    for rt in range(RT):
                        for pr in range(9):
                            nc.tensor.matmul(
                                pts[rt][:], wa_v[:, pr],
                                _pair_rhs(xa_v, xb_v, rt, pr),
                                start=(pr == 0), stop=False,
                                perf_mode=mybir.MatmulPerfMode.DoubleRow,
                            )
                    # phase 2: pairs 9-13 (q2 planes); pr12 closes the group
                    for rt in range(RT):
                        for pr in (9, 10, 11, 13, 12):
                            nc.tensor.matmul(
                                pts[rt][:], wb_v[:, pr - 9],
                                _pair_rhs(xa_v, xb_v, rt, pr),
                                start=False, stop=(pr == 12),
                                perf_mode=mybir.MatmulPerfMode.DoubleRow,
                            )
                        col = img * RT + rt
                        zs = z[j][:, img * PIX + rt * NTILE
                                  : img * PIX + (rt + 1) * NTILE]
                        nc.vector.tensor_scalar(
                            out=zs, in0=pts[rt][:], scalar1=0.0, scalar2=None,
                            op0=mybir.AluOpType.add, op1=mybir.AluOpType.add,
                            accum_out=ssum[:, j * npart + col
                                           : j * npart + col + 1],
                        )
                        sqt = sqp.tile([128, NTILE], F32, tag="sqt")
                        nc.vector.scalar_tensor_tensor(
                            out=sqt[:], in0=pts[rt][:], scalar=1.0, in1=zs,
                            op0=mybir.AluOpType.mult, op1=mybir.AluOpType.mult,
                            accum_out=ssq[:, j * npart + col
                                          : j * npart + col + 1],
                        )

                # ---- chunk-j stats on Pool: [128,2] = (sum, sumsq) ----
                cc_sb = sp.tile([128, 2], F32, tag=f"ccsb{j}", name=f"ccsb{j}")
                nc.vector.reduce_sum(
                    out=cc_sb[:, 0:1], in_=ssum[:, j * npart : (j + 1) * npart],
                    axis=mybir.AxisListType.X,
                )
                nc.vector.reduce_sum(
                    out=cc_sb[:, 1:2], in_=ssq[:, j * npart : (j + 1) * npart],
                    axis=mybir.AxisListType.X,
                )
                st = sp.tile([128, 2], F32, tag=f"st{j}", name=f"st{j}")
                if collective and n_cores > 1:
                    cc_in = dp.tile([128, 2], F32, tag=f"ccin{j}",
                                    name=f"ccin{j}")
                    cc_out = dp.tile([128, 2], F32, tag=f"ccout{j}",
                                     name=f"ccout{j}")
                    nc.sync.dma_start(cc_in[:], cc_sb[:])
                    nc.gpsimd.collective_compute(
                        "AllReduce", mybir.AluOpType.add,
                        replica_groups=[list(range(n_cores))],
                        ins=[cc_in.opt()], outs=[cc_out.opt()],
                    )
                    nc.sync.dma_start(st[:], cc_out[:])
                else:
                    nc.gpsimd.tensor_copy(st[:], cc_sb[:])

                # ---- BN affine:  A = R*rsqrt(P*q - Qc*s^2 + eps) >= 0,
                #      B = beta - mu*A,  T = mu - beta/A  (tail chunk only).
                # Chunk 0's math runs on Pool (DVE is busy with tile
                # consumers of chunk 1); the tail chunk's runs on DVE.
                last = j == NCHUNK - 1
                eng = nc.vector if last else nc.gpsimd
                Pj, Qj, Rj, bj = (v[:, j : j + 1] for v in (P_, Qc, R_, beta))
                s0, s1 = st[:, 0:1], st[:, 1:2]
                mu = sp.tile([128, 1], F32, tag=f"mu{j}", name=f"mu{j}")
                u = sp.tile([128, 1], F32, tag=f"u{j}", name=f"u{j}")
                A = sp.tile([128, 1], F32, tag=f"A{j}", name=f"A{j}")
                B = sp.tile([128, 1], F32, tag=f"B{j}", name=f"B{j}")
                t1 = sp.tile([128, 1], F32, tag=f"t1{j}", name=f"t1{j}")

                eng.tensor_tensor(out=u[:], in0=s1, in1=Pj,
                                  op=mybir.AluOpType.mult)
                eng.tensor_tensor(out=t1[:], in0=s0, in1=s0,
                                  op=mybir.AluOpType.mult)
                eng.tensor_tensor(out=t1[:], in0=t1[:], in1=Qj,
                                  op=mybir.AluOpType.mult)
                eng.tensor_tensor(out=u[:], in0=u[:], in1=t1[:],
                                  op=mybir.AluOpType.subtract)
                eng.tensor_scalar(out=u[:], in0=u[:], scalar1=float(BN_EPS),
                                  scalar2=None, op0=mybir.AluOpType.add)
                nc.vector.reciprocal(u[:], u[:])
                nc.scalar.activation(out=u[:], in_=u[:],
                                     func=mybir.ActivationFunctionType.Sqrt)
                eng.tensor_tensor(out=A[:], in0=Rj, in1=u[:],
                                  op=mybir.AluOpType.mult)
                eng.tensor_scalar(out=mu[:], in0=s0, scalar1=inv_n,
                                  scalar2=None, op0=mybir.AluOpType.mult)
                eng.tensor_tensor(out=t1[:], in0=mu[:], in1=A[:],
                                  op=mybir.AluOpType.mult)
                eng.tensor_tensor(out=B[:], in0=bj, in1=t1[:],
                                  op=mybir.AluOpType.subtract)

                if not last:
                    # ---- chunk 0: ACT signs all 4 images (overlaps chunk-1
                    # conv; ACT has no other work) ----
                    for img in range(IMGS):
                        ostg = op_.tile([128, PIX], BF16, tag="ostg",
                                        name=f"ostg{j}_{img}")
                        nc.scalar.activation(
                            out=ostg[:],
                            in_=z[j][:, img * PIX : (img + 1) * PIX],
                            func=mybir.ActivationFunctionType.Sign,
                            bias=B[:, 0:1], scale=A[:, 0:1],
                        )
                        nc.sync.dma_start(out_d[img, j], ostg[:])
                    continue

                # ---- tail chunk: split signs ACT / DVE / Pool ----
                T = sp.tile([128, 1], F32, tag="T", name="T")
                nc.vector.reciprocal(t1[:], A[:])
                nc.vector.tensor_tensor(out=t1[:], in0=bj, in1=t1[:],
                                        op=mybir.AluOpType.mult)
                nc.vector.tensor_tensor(out=T[:], in0=mu[:], in1=t1[:],
                                        op=mybir.AluOpType.subtract)

                HFX = PIX // 2
                ost = [op_.tile([128, PIX], BF16, tag="ostg",
                                name=f"ostg{j}_{img}") for img in range(IMGS)]
                # ACT: imgs 0, 1 and first half of img 2 (Sign activation)
                for img in range(2):
                    nc.scalar.activation(
                        out=ost[img][:],
                        in_=z[j][:, img * PIX : (img + 1) * PIX],
                        func=mybir.ActivationFunctionType.Sign,
                        bias=B[:, 0:1], scale=A[:, 0:1],
                    )
                    nc.sync.dma_start(out_d[img, j], ost[img][:])
                nc.scalar.activation(
                    out=ost[2][:, 0:HFX],
                    in_=z[j][:, 2 * PIX : 2 * PIX + HFX],
                    func=mybir.ActivationFunctionType.Sign,
                    bias=B[:, 0:1], scale=A[:, 0:1],
                )
                # DVE: second half of img 2 (compare to T, then 2x-1)
                ph = z[j][:, 2 * PIX + HFX : 3 * PIX]
                nc.vector.tensor_tensor(
                    out=ost[2][:, HFX:PIX], in0=ph,
                    in1=T[:, 0:1].broadcast_to([128, HFX]),
                    op=mybir.AluOpType.is_ge,
                )
                nc.vector.tensor_scalar(
                    out=ost[2][:, HFX:PIX], in0=ost[2][:, HFX:PIX],
                    scalar1=2.0, scalar2=-1.0,
                    op0=mybir.AluOpType.mult, op1=mybir.AluOpType.add,
                )
                nc.sync.dma_start(out_d[2, j], ost[2][:])
                # DVE: img 3 (compare to T, then 2x-1)
                nc.vector.tensor_tensor(
                    out=ost[3][:], in0=z[j][:, 3 * PIX : 4 * PIX],
                    in1=T[:, 0:1].broadcast_to([128, PIX]),
                    op=mybir.AluOpType.is_ge,
                )
                nc.vector.tensor_scalar(
                    out=ost[3][:], in0=ost[3][:],
                    scalar1=2.0, scalar2=-1.0,
                    op0=mybir.AluOpType.mult, op1=mybir.AluOpType.add,
                )
                nc.sync.dma_start(out_d[3, j], ost[3][:])

    _split_multi_waits(nc)
    return nc


def _prep_inputs(x, weight, gamma, beta):
    """Host-side prep: sign/alpha/gamma folding, padding, 3-term fp8 split."""
    x = np.ascontiguousarray(x, dtype=np.float32)
    weight = np.ascontiguousarray(weight, dtype=np.float32)
    gamma = np.asarray(gamma, np.float32)
    beta = np.asarray(beta, np.float32)

    alpha = np.abs(weight).mean(axis=(1, 2, 3)).astype(np.float32)      # [256]
    S = np.where(gamma >= 0, np.float32(1), np.float32(-1))
    sgn = np.where(weight >= 0, np.float32(1), np.float32(-1)) * S[:, None, None, None]

    # ---- fp8 weight pairs: wq[cin, j, pair, ktile, o] ----
    sgn_t = sgn.transpose(1, 2, 3, 0).reshape(CIN, KH * KW, NCHUNK, 128)
    wq = np.zeros((CIN, NCHUNK, NPAIR, 2, 128), np.float32)
    S1, S2 = 2.0 ** -4, 2.0 ** -6
    for k in range(9):                      # pairs 0-8: (q0, q1) of tap k
        wq[:, :, k, 0] = sgn_t[:, k]
        wq[:, :, k, 1] = sgn_t[:, k] * S1
    for dy in range(3):                     # pairs 9-11: q2 taps (dy,0)+(dy,1)
        wq[:, :, 9 + dy, 0] = sgn_t[:, dy * 3 + 0] * S2
        wq[:, :, 9 + dy, 1] = sgn_t[:, dy * 3 + 1] * S2
    wq[:, :, 12, 0] = sgn_t[:, 2] * S2      # pair 12: q2 taps (0,2)+(1,2)
    wq[:, :, 12, 1] = sgn_t[:, 5] * S2
    wq[:, :, 13, 0] = sgn_t[:, 8] * S2      # pair 13: q2 tap (2,2) + zeros
    wq = np.ascontiguousarray(
        wq.reshape(CIN, NCHUNK * NPAIR * WSLOT)
    ).astype(NP8)

    # pqrb[p, j]: P = a^2/N | Qc = a^2/N^2 | R = a*|g| | beta
    def chunked(v):
        return np.ascontiguousarray(v.reshape(NCHUNK, 128).T)  # [128, 2]
    a2 = alpha * alpha
    pqrb = np.concatenate(
        [chunked(a2 / NTOT), chunked(a2 / NTOT / NTOT),
         chunked(alpha * np.abs(gamma)), chunked(beta)], axis=1
    ).astype(np.float32)                                                # [128, 8]

    # ---- 3-term fp8 split of padded x, with shifted q2 planes ----
    xpad = np.zeros((N_FULL, CIN, HP, WP), np.float32)
    xpad[:, :, 1 : H + 1, 1 : W + 1] = x
    xpad = xpad.reshape(N_FULL, CIN, PADPIX)
    q0 = xpad.astype(NP8)
    r1 = xpad - q0.astype(np.float32)
    q1 = (r1 * 16.0).astype(NP8)
    r2 = r1 - q1.astype(np.float32) * (1.0 / 16.0)
    q2 = (r2 * 64.0).astype(NP8)
    q2p = np.zeros((N_FULL, CIN, PADPIX + 64), NP8)
    q2p[:, :, :PADPIX] = q2
    xq = np.stack(
        [q0, q1, q2, q2p[:, :, 1 : 1 + PADPIX], q2p[:, :, 59 : 59 + PADPIX]],
        axis=2,
    )                                                   # [N, CIN, 5, PADPIX]
    xq = np.ascontiguousarray(xq.reshape(N_FULL, CIN, 5 * PADPIX))

    in_maps = []
    for c in range(N_CORES):
        sl = slice(c * IMGS, (c + 1) * IMGS)
        in_maps.append({
            "xq": np.ascontiguousarray(xq[sl]),
            "wq": wq,
            "pqrb": pqrb,
        })
    return in_maps


def kernel(x, weight, gamma, beta):
    in_maps = _prep_inputs(x, weight, gamma, beta)
    nc = build_bass()
    res = run_bass_kernel_spmd(nc, in_maps, core_ids=list(range(N_CORES)))
    out = np.empty((N_FULL, COUT, H, W), np.float32)
    for c in range(N_CORES):
        o = res.results[c]["out"]          # [IMGS, 2, 128, 3136] bf16 (+-1)
        o = o.astype(np.float32).reshape(IMGS, COUT, H, W)
        out[c * IMGS : (c + 1) * IMGS] = o
    return out


# revision 13
# speedup vs baseline: 2.1872x; 1.0806x over previous
"""Trainium2 Bass kernel for ConvBnSign (binarized 3x3 conv + sync-BN + sign).

Math: y = conv2d(x, sign(w) * alpha)  with alpha = mean|w| per out-channel,
then train-mode BatchNorm over (N,H,W), then hard_sign.

Folds: alpha > 0 folds into the BN affine; S = sign(gamma) folds into the
binarized weights (z' = S*z), making the BN scale A = alpha*|gamma|*rsqrt(
alpha^2 var + eps) >= 0 so the final sign is also a per-channel threshold
compare  out = (z' >= T) ? +1 : -1,  T = mu' - beta/A  — which lets the
tail's sign pass split across ACT / DVE / Pool engines.

Precision: x is split on host into 3 fp8-e4m3 terms
  x ~ q0 + q1/16 + q2/64   (residual rms ~1.6e-5 relative),
with per-term scales folded into the fp8 weights (1, 2^-4, 2^-6 — all
normal in e4m3). Each conv tile is 14 DoubleRow fp8 matmuls (2 k-tiles
each, 0.5 cyc/row): the 27 (tap, term) k-tiles pair via the plane dim of
the SBUF layout, where two extra planes are host-shifted copies of q2
(shift +1 / +59) so cross-tap pairs land at the uniform plane stride.

Per-tile stats consumers both run on DVE (tensor_scalar z-copy+sum and
tensor_tensor_reduce square+sum), keeping ACT free for Sign and Pool free
for reduces/BN math — avoids ACT head-of-line blocking on PSUM drains.

Sharding: data-parallel, 4 images per core across 8 cores; BN stats are
per-channel partial sums [128,4] fp32 all-reduced across cores.
"""

import numpy as np
import ml_dtypes

import concourse.bass as bass
import concourse.mybir as mybir
import concourse.tile as tile
from concourse.vector_clock import ScopedClock
from concourse.bass_utils import run_bass_kernel_spmd

# ---- problem constants (hardcoded per contract) ----
N_CORES = 8
N_FULL = 32           # batch
CIN = 128             # input channels
COUT = 256            # output channels
H = W = 56
KH = KW = 3
BN_EPS = 1e-5

IMGS = N_FULL // N_CORES          # 4 images per core
WP = W + 2                        # 58 padded width
HP = H + 2
PADPIX = HP * WP                  # 3364
PIX = H * W                       # 3136
NCHUNK = COUT // 128              # 2 chunks of 128 output channels
RTR = 8                           # rows per matmul tile
RT = H // RTR                     # 7 row tiles per image
NTILE = RTR * W                   # 448 = matmul free dim (<=512, one PSUM bank)
NTOT = N_FULL * PIX               # 200704 elements per channel for BN stats
NPAIR = 14                        # DoubleRow matmuls per PSUM tile
HFX = PIX // 2                    # half-image columns (sign/DMA granularity)
WSLOT = 2 * 128                   # fp8 bytes per weight pair
PHA = 9 * WSLOT                   # phase-1 weight bytes (pairs 0-8)
PHB = 5 * WSLOT                   # phase-2 weight bytes (pairs 9-13)

BF16 = mybir.dt.bfloat16
F32 = mybir.dt.float32
FP8 = mybir.dt.float8e4
NP8 = ml_dtypes.float8_e4m3

_MAX_DRAIN_WAITS = 1  # walrus CTRL instructions accept a single sync wait


def _split_multi_waits(nc, max_waits=1):
    """This walrus build rejects instructions with more than one sem wait.
    Hoist excess waits onto same-engine NoOps inserted immediately before the
    offending instruction (the engine blocks at the NoOp instead — identical
    ordering semantics)."""
    ctr = 0
    for bbw in nc.main_func.blocks:
        out = []
        changed = False
        for inst in bbw.instructions:
            si = inst.sync_info
            w = list(si.on_wait or []) if si else []
            if len(w) > max_waits:
                changed = True
                excess = w[: len(w) - max_waits]
                for i in range(0, len(excess), max_waits):
                    nop = mybir.InstNoOp(name=f"WFIX-{ctr}", ins=[], outs=[])
                    ctr += 1
                    nop.engine = inst.engine
                    nop.sync_info = mybir.SyncInfo(
                        on_wait=excess[i : i + max_waits], on_update=[]
                    )
                    out.append(nop)
                inst.sync_info = mybir.SyncInfo(
                    on_wait=w[len(w) - max_waits :],
                    on_update=list(si.on_update or []),
                )
            out.append(inst)
        if changed:
            bbw.instructions = out
    return ctr


class _SplitDrainTileContext(tile.TileContext):
    """TileContext whose final drain splits its sem waits across multiple
    sync-engine instructions (this walrus build caps CTRL waits at 1)."""

    def _drain_and_barrier(self, tick_clock, wait_clock):
        drain_inst = self.nc.sync.drain()
        wait_clock.add_sem_waits(
            drain_inst.ins, ScopedClock({None: tick_clock.global_clock})
        )
        si = drain_inst.ins.sync_info
        w = list(si.on_wait or [])
        if len(w) > _MAX_DRAIN_WAITS:
            drain_inst.ins.sync_info = mybir.SyncInfo(
                on_wait=w[:_MAX_DRAIN_WAITS], on_update=list(si.on_update or [])
            )
            for i in range(_MAX_DRAIN_WAITS, len(w), _MAX_DRAIN_WAITS):
                nop = self.nc.sync.nop(nofuse=True)
                nop.ins.sync_info = mybir.SyncInfo(
                    on_wait=w[i : i + _MAX_DRAIN_WAITS], on_update=[]
                )
        self.nc.all_engine_barrier()
        assert self.sems is not None
        popped = self.nc._tile_sem_poison_stack.pop()
        assert popped is self._sem_poison
        self.nc.clear_and_free_semaphores(list(self.sems.allocated().values()))
        self.nc.all_engine_barrier()


def _pair_rhs(xa_v, xb_v, rt, pr):
    """rhs AP [128, 2, 8, 56] for DoubleRow pair pr of row-tile rt.

    xa_v: [128, 2, HP, WP] view of planes (q0, q1);
    xb_v: [128, 3, HP, WP] view of planes (q2, q2<<1, q2<<59)."""
    r0 = rt * RTR
    if pr < 9:                       # (tap pr, q0) + (tap pr, q1)
        dy, dx = divmod(pr, KW)
        return xa_v[:, 0:2, r0 + dy : r0 + dy + RTR, dx : dx + W]
    if pr < 12:                      # q2 taps (dy,0)+(dy,1) via shift-1 plane
        dy = pr - 9
        return xb_v[:, 0:2, r0 + dy : r0 + dy + RTR, 0:W]
    if pr == 12:                     # q2 taps (0,2)+(1,2) via shift-59 plane
        return xb_v[:, 1:3, r0 : r0 + RTR, 1 : 1 + W]
    # pr == 13: q2 tap (2,2), second half has zero weights
    return xb_v[:, 0:1, r0 + 2 : r0 + 2 + RTR, 2 : 2 + W].broadcast_to(
        [128, 2, RTR, W]
    )


def build_bass(n_cores=N_CORES, collective=True):
    """Build the per-core Bass module (SPMD: same program on every core)."""
    nc = bass.Bass(num_devices=n_cores)

    xq_d = nc.dram_tensor("xq", [IMGS, CIN, 5 * PADPIX], FP8,
                          kind="ExternalInput")
    wq_d = nc.dram_tensor("wq", [CIN, NCHUNK * NPAIR * WSLOT], FP8,
                          kind="ExternalInput")
    # pqrb[p, j-col chunks]: P | Qc | R | beta  (4 cols per chunk)
    pqrb_d = nc.dram_tensor("pqrb", [128, 4 * NCHUNK], F32,
                            kind="ExternalInput")
    out_d = nc.dram_tensor("out", [IMGS, NCHUNK, 128, PIX], BF16,
                           kind="ExternalOutput")

    with _SplitDrainTileContext(nc) as tc:
        with (
            tc.tile_pool(name="const", bufs=1) as constp,
            tc.tile_pool(name="xbuf", bufs=1) as xp,
            tc.tile_pool(name="zbuf", bufs=1) as zp,
            tc.tile_pool(name="stats", bufs=1) as sp,
            tc.tile_pool(name="sq", bufs=2) as sqp,
            tc.tile_pool(name="ost", bufs=6) as op_,
            tc.tile_pool(name="pz", bufs=8, space="PSUM") as pp,
            tc.tile_pool(name="dram", bufs=1, space="DRAM") as dp,
        ):
            # ---- weights (split per chunk/phase for early PE start) ----
            wa = [constp.tile([128, PHA], FP8, tag=f"wa{j}", name=f"wa{j}")
                  for j in range(NCHUNK)]
            wb = [constp.tile([128, PHB], FP8, tag=f"wb{j}", name=f"wb{j}")
                  for j in range(NCHUNK)]
            pqrb_sb = constp.tile([128, 4 * NCHUNK], F32, tag="pqrb")

            # ---- x plane stacks (per image, split planes 01 / 234) ----
            xa = [xp.tile([128, 2 * PADPIX], FP8, tag=f"xa{i}", name=f"xa{i}")
                  for i in range(IMGS)]
            xb = [xp.tile([128, 3 * PADPIX], FP8, tag=f"xb{i}", name=f"xb{i}")
                  for i in range(IMGS)]

            # DMA issue order = arrival order: w0a, img0 planes, w0b, then
            # the rest.  First matmul waits only on w0a + xa[0].
            nc.sync.dma_start(wa[0][:], wq_d[:, 0:PHA])
            nc.sync.dma_start(xa[0][:], xq_d[0][:, 0 : 2 * PADPIX])
            nc.sync.dma_start(wb[0][:], wq_d[:, PHA : PHA + PHB])
            nc.sync.dma_start(xb[0][:], xq_d[0][:, 2 * PADPIX :])
            nc.sync.dma_start(pqrb_sb[:], pqrb_d[:])
            for i in range(1, IMGS):
                nc.sync.dma_start(xa[i][:], xq_d[i][:, 0 : 2 * PADPIX])
                nc.sync.dma_start(xb[i][:], xq_d[i][:, 2 * PADPIX :])
            off = NPAIR * WSLOT
            nc.sync.dma_start(wa[1][:], wq_d[:, off : off + PHA])
            nc.sync.dma_start(wb[1][:], wq_d[:, off + PHA : off + PHA + PHB])

            # ---- z buffers + stats ----
            z = [zp.tile([128, IMGS * PIX], F32, tag=f"z{j}", name=f"z{j}")
                 for j in range(NCHUNK)]
            ssum = sp.tile([128, 64], F32, tag="ssum")
            ssq = sp.tile([128, 64], F32, tag="ssq")

            P_ = pqrb_sb[:, 0:NCHUNK]
            Qc = pqrb_sb[:, NCHUNK : 2 * NCHUNK]
            R_ = pqrb_sb[:, 2 * NCHUNK : 3 * NCHUNK]
            beta = pqrb_sb[:, 3 * NCHUNK : 4 * NCHUNK]
            inv_n = 1.0 / NTOT
            npart = IMGS * RT

            def _consume(j, img, rt, pt):
                """DVE stats consumers for one finished PSUM tile: z-copy
                with sum accumulation, then square with sumsq accumulation."""
                col = img * RT + rt
                zs = z[j][:, img * PIX + rt * NTILE
                          : img * PIX + (rt + 1) * NTILE]
                nc.vector.tensor_scalar(
                    out=zs, in0=pt[:], scalar1=0.0, scalar2=None,
                    op0=mybir.AluOpType.add, op1=mybir.AluOpType.add,
                    accum_out=ssum[:, j * npart + col : j * npart + col + 1],
                )
                sqt = sqp.tile([128, NTILE], F32, tag="sqt")
                nc.vector.scalar_tensor_tensor(
                    out=sqt[:], in0=pt[:], scalar=1.0, in1=zs,
                    op0=mybir.AluOpType.mult, op1=mybir.AluOpType.mult,
                    accum_out=ssq[:, j * npart + col : j * npart + col + 1],
                )

            def _mm(pt, wv, xa_v, xb_v, rt, pr):
                nc.tensor.matmul(
                    pt[:], wv[:, pr] if pr < 9 else wv[:, pr - 9],
                    _pair_rhs(xa_v, xb_v, rt, pr),
                    start=(pr == 0), stop=(pr == 12),
                    perf_mode=mybir.MatmulPerfMode.DoubleRow,
                )

            TILE_ORDER = list(range(9)) + [9, 10, 11, 13, 12]

            for j in range(NCHUNK):
                wa_v = wa[j][:].rearrange("p (pr k o) -> p pr k o", pr=9, k=2)
                wb_v = wb[j][:].rearrange("p (pr k o) -> p pr k o", pr=5, k=2)

                for img in range(IMGS):
                    xa_v = xa[img][:].rearrange("p (t r c) -> p t r c",
                                                t=2, r=HP)
                    xb_v = xb[img][:].rearrange("p (t r c) -> p t r c",
                                                t=3, r=HP)
                    pts = [pp.tile([128, NTILE], F32, tag="pz",
                                   name=f"pz{j}_{img}_{rt}")
                           for rt in range(RT)]
                    if j == 0 and img == 0:
                        # Phased: pairs 0-8 (planes q0/q1, early DMA) across
                        # all row tiles first — a 63-matmul runway while
                        # xb[0] is still in flight.
                        for rt in range(RT):
                            for pr in range(9):
                                _mm(pts[rt], wa_v, xa_v, xb_v, rt, pr)
                        for rt in range(RT):
                            for pr in (9, 10, 11, 13, 12):
                                _mm(pts[rt], wb_v, xa_v, xb_v, rt, pr)
                            _consume(j, img, rt, pts[rt])
                    else:
                        # Tile-major: each tile's 14 pairs are consecutive so
                        # group-closures are spaced a full tile apart and the
                        # DVE consumers (~1184 ns/tile < 1306 ns/tile) keep up.
                        for rt in range(RT):
                            for pr in TILE_ORDER:
                                _mm(pts[rt], wa_v if pr < 9 else wb_v,
                                    xa_v, xb_v, rt, pr)
                            _consume(j, img, rt, pts[rt])

                # ---- chunk-j stats on Pool: [128,2] = (sum, sumsq) ----
                cc_sb = sp.tile([128, 2], F32, tag=f"ccsb{j}", name=f"ccsb{j}")
                nc.vector.reduce_sum(
                    out=cc_sb[:, 0:1], in_=ssum[:, j * npart : (j + 1) * npart],
                    axis=mybir.AxisListType.X,
                )
                nc.vector.reduce_sum(
                    out=cc_sb[:, 1:2], in_=ssq[:, j * npart : (j + 1) * npart],
                    axis=mybir.AxisListType.X,
                )
                st = sp.tile([128, 2], F32, tag=f"st{j}", name=f"st{j}")
                if collective and n_cores > 1:
                    cc_in = dp.tile([128, 2], F32, tag=f"ccin{j}",
                                    name=f"ccin{j}")
                    cc_out = dp.tile([128, 2], F32, tag=f"ccout{j}",
                                     name=f"ccout{j}")
                    nc.sync.dma_start(cc_in[:], cc_sb[:])
                    nc.gpsimd.collective_compute(
                        "AllReduce", mybir.AluOpType.add,
                        replica_groups=[list(range(n_cores))],
                        ins=[cc_in.opt()], outs=[cc_out.opt()],
                    )
                    nc.sync.dma_start(st[:], cc_out[:])
                else:
                    nc.gpsimd.tensor_copy(st[:], cc_sb[:])

                # ---- BN affine:  A = R*rsqrt(P*q - Qc*s^2 + eps) >= 0,
                #      B = beta - mu*A,  T = mu - beta/A  (tail chunk only).
                # Chunk 0's math runs on Pool (DVE is busy with tile
                # consumers of chunk 1); the tail chunk's runs on DVE.
                last = j == NCHUNK - 1
                eng = nc.vector if last else nc.gpsimd
                Pj, Qj, Rj, bj = (v[:, j : j + 1] for v in (P_, Qc, R_, beta))
                s0, s1 = st[:, 0:1], st[:, 1:2]
                mu = sp.tile([128, 1], F32, tag=f"mu{j}", name=f"mu{j}")
                u = sp.tile([128, 1], F32, tag=f"u{j}", name=f"u{j}")
                A = sp.tile([128, 1], F32, tag=f"A{j}", name=f"A{j}")
                B = sp.tile([128, 1], F32, tag=f"B{j}", name=f"B{j}")
                t1 = sp.tile([128, 1], F32, tag=f"t1{j}", name=f"t1{j}")

                eng.tensor_tensor(out=u[:], in0=s1, in1=Pj,
                                  op=mybir.AluOpType.mult)
                eng.tensor_tensor(out=t1[:], in0=s0, in1=s0,
                                  op=mybir.AluOpType.mult)
                eng.tensor_tensor(out=t1[:], in0=t1[:], in1=Qj,
                                  op=mybir.AluOpType.mult)
                eng.tensor_tensor(out=u[:], in0=u[:], in1=t1[:],
                                  op=mybir.AluOpType.subtract)
                eng.tensor_scalar(out=u[:], in0=u[:], scalar1=float(BN_EPS),
                                  scalar2=None, op0=mybir.AluOpType.add)
                nc.vector.reciprocal(u[:], u[:])
                nc.scalar.activation(out=u[:], in_=u[:],
                                     func=mybir.ActivationFunctionType.Sqrt)
                eng.tensor_tensor(out=A[:], in0=Rj, in1=u[:],
                                  op=mybir.AluOpType.mult)
                eng.tensor_scalar(out=mu[:], in0=s0, scalar1=inv_n,
                                  scalar2=None, op0=mybir.AluOpType.mult)
                eng.tensor_tensor(out=t1[:], in0=mu[:], in1=A[:],
                                  op=mybir.AluOpType.mult)
                eng.tensor_tensor(out=B[:], in0=bj, in1=t1[:],
                                  op=mybir.AluOpType.subtract)

                def _act_sign(img, h, tag):
                    """ACT Sign on half-image h of img -> staged -> DRAM."""
                    o = op_.tile([128, HFX], BF16, tag="ostg", name=tag)
                    lo = img * PIX + h * HFX
                    nc.scalar.activation(
                        out=o[:], in_=z[j][:, lo : lo + HFX],
                        func=mybir.ActivationFunctionType.Sign,
                        bias=B[:, 0:1], scale=A[:, 0:1],
                    )
                    nc.sync.dma_start(
                        out_d[img, j][:, h * HFX : (h + 1) * HFX], o[:])

                if not last:
                    # ---- chunk 0: ACT signs all 4 images as 8 half-image
                    # passes (overlaps chunk-1 conv; ACT has no other work) --
                    for img in range(IMGS):
                        for h in range(2):
                            _act_sign(img, h, f"ostg{j}_{img}_{h}")
                    continue

                # ---- tail chunk: ACT takes imgs 0-1, DVE takes imgs 2-3
                # as single-pass compares (z >= T) -> {1,0}; the host maps
                # {1,0} -> {+1,-1} for exactly these slices. ----
                T = sp.tile([128, 1], F32, tag="T", name="T")
                nc.vector.reciprocal(t1[:], A[:])
                nc.vector.tensor_tensor(out=t1[:], in0=bj, in1=t1[:],
                                        op=mybir.AluOpType.mult)
                nc.vector.tensor_tensor(out=T[:], in0=mu[:], in1=t1[:],
                                        op=mybir.AluOpType.subtract)

                for img in (2, 3):
                    for h in range(2):
                        o = op_.tile([128, HFX], BF16, tag="ostg",
                                     name=f"ostg{j}_{img}_{h}")
                        lo = img * PIX + h * HFX
                        nc.vector.tensor_tensor(
                            out=o[:], in0=z[j][:, lo : lo + HFX],
                            in1=T[:, 0:1].broadcast_to([128, HFX]),
                            op=mybir.AluOpType.is_ge,
                        )
                        nc.sync.dma_start(
                            out_d[img, j][:, h * HFX : (h + 1) * HFX], o[:])
                for img in (0, 1):
                    for h in range(2):
                        _act_sign(img, h, f"ostg{j}_{img}_{h}")

    _split_multi_waits(nc)
    return nc


def _prep_inputs(x, weight, gamma, beta):
    """Host-side prep: sign/alpha/gamma folding, padding, 3-term fp8 split."""
    x = np.ascontiguousarray(x, dtype=np.float32)
    weight = np.ascontiguousarray(weight, dtype=np.float32)
    gamma = np.asarray(gamma, np.float32)
    beta = np.asarray(beta, np.float32)

    alpha = np.abs(weight).mean(axis=(1, 2, 3)).astype(np.float32)      # [256]
    S = np.where(gamma >= 0, np.float32(1), np.float32(-1))
    sgn = np.where(weight >= 0, np.float32(1), np.float32(-1)) * S[:, None, None, None]

    # ---- fp8 weight pairs: wq[cin, j, pair, ktile, o] ----
    sgn_t = sgn.transpose(1, 2, 3, 0).reshape(CIN, KH * KW, NCHUNK, 128)
    wq = np.zeros((CIN, NCHUNK, NPAIR, 2, 128), np.float32)
    S1, S2 = 2.0 ** -4, 2.0 ** -6
    for k in range(9):                      # pairs 0-8: (q0, q1) of tap k
        wq[:, :, k, 0] = sgn_t[:, k]
        wq[:, :, k, 1] = sgn_t[:, k] * S1
    for dy in range(3):                     # pairs 9-11: q2 taps (dy,0)+(dy,1)
        wq[:, :, 9 + dy, 0] = sgn_t[:, dy * 3 + 0] * S2
        wq[:, :, 9 + dy, 1] = sgn_t[:, dy * 3 + 1] * S2
    wq[:, :, 12, 0] = sgn_t[:, 2] * S2      # pair 12: q2 taps (0,2)+(1,2)
    wq[:, :, 12, 1] = sgn_t[:, 5] * S2
    wq[:, :, 13, 0] = sgn_t[:, 8] * S2      # pair 13: q2 tap (2,2) + zeros
    wq = np.ascontiguousarray(
        wq.reshape(CIN, NCHUNK * NPAIR * WSLOT)
    ).astype(NP8)

    # pqrb[p, j]: P = a^2/N | Qc = a^2/N^2 | R = a*|g| | beta
    def chunked(v):
        return np.ascontiguousarray(v.reshape(NCHUNK, 128).T)  # [128, 2]
    a2 = alpha * alpha
    pqrb = np.concatenate(
        [chunked(a2 / NTOT), chunked(a2 / NTOT / NTOT),
         chunked(alpha * np.abs(gamma)), chunked(beta)], axis=1
    ).astype(np.float32)                                                # [128, 8]

    # ---- 3-term fp8 split of padded x, with shifted q2 planes ----
    xpad = np.zeros((N_FULL, CIN, HP, WP), np.float32)
    xpad[:, :, 1 : H + 1, 1 : W + 1] = x
    xpad = xpad.reshape(N_FULL, CIN, PADPIX)
    q0 = xpad.astype(NP8)
    r1 = xpad - q0.astype(np.float32)
    q1 = (r1 * 16.0).astype(NP8)
    r2 = r1 - q1.astype(np.float32) * (1.0 / 16.0)
    q2 = (r2 * 64.0).astype(NP8)
    q2p = np.zeros((N_FULL, CIN, PADPIX + 64), NP8)
    q2p[:, :, :PADPIX] = q2
    xq = np.stack(
        [q0, q1, q2, q2p[:, :, 1 : 1 + PADPIX], q2p[:, :, 59 : 59 + PADPIX]],
        axis=2,
    )                                                   # [N, CIN, 5, PADPIX]
    xq = np.ascontiguousarray(xq.reshape(N_FULL, CIN, 5 * PADPIX))

    in_maps = []
    for c in range(N_CORES):
        sl = slice(c * IMGS, (c + 1) * IMGS)
        in_maps.append({
            "xq": np.ascontiguousarray(xq[sl]),
            "wq": wq,
            "pqrb": pqrb,
        })
    return in_maps


def kernel(x, weight, gamma, beta):
    in_maps = _prep_inputs(x, weight, gamma, beta)
    nc = build_bass()
    res = run_bass_kernel_spmd(nc, in_maps, core_ids=list(range(N_CORES)))
    out = np.empty((N_FULL, COUT, H, W), np.float32)
    for c in range(N_CORES):
        o = res.results[c]["out"].astype(np.float32)   # [IMGS, 2, 128, PIX]
        o[2:4, 1] = o[2:4, 1] * 2.0 - 1.0   # DVE compare slices: {1,0}->{+-1}
        out[c * IMGS : (c + 1) * IMGS] = o.reshape(IMGS, COUT, H, W)
    return out


# revision 14
# speedup vs baseline: 2.2884x; 1.0462x over previous
"""Trainium2 Bass kernel for ConvBnSign (binarized 3x3 conv + sync-BN + sign).

Math: y = conv2d(x, sign(w) * alpha)  with alpha = mean|w| per out-channel,
then train-mode BatchNorm over (N,H,W), then hard_sign.

Folds: alpha > 0 folds into the BN affine; S = sign(gamma) folds into the
binarized weights (z' = S*z), making the BN scale A = alpha*|gamma|*rsqrt(
alpha^2 var + eps) >= 0 so the final sign is also a per-channel threshold
compare  out = (z' >= T) ? +1 : -1,  T = mu' - beta/A  — which lets the
tail's sign pass split across ACT / DVE / Pool engines.

Precision: x is split on host into 3 fp8-e4m3 terms
  x ~ q0 + q1/16 + q2/64   (residual rms ~1.6e-5 relative),
with per-term scales folded into the fp8 weights (1, 2^-4, 2^-6 — all
normal in e4m3). Each conv tile is 14 DoubleRow fp8 matmuls (2 k-tiles
each, 0.5 cyc/row): the 27 (tap, term) k-tiles pair via the plane dim of
the SBUF layout, where two extra planes are host-shifted copies of q2
(shift +1 / +59) so cross-tap pairs land at the uniform plane stride.

Per-tile stats consumers both run on DVE (tensor_scalar z-copy+sum and
tensor_tensor_reduce square+sum), keeping ACT free for Sign and Pool free
for reduces/BN math — avoids ACT head-of-line blocking on PSUM drains.

Sharding: data-parallel, 4 images per core across 8 cores; BN stats are
per-channel partial sums [128,4] fp32 all-reduced across cores.
"""

import numpy as np
import ml_dtypes

import concourse.bass as bass
import concourse.mybir as mybir
import concourse.tile as tile
from concourse.vector_clock import ScopedClock
from concourse.bass_utils import run_bass_kernel_spmd

# ---- problem constants (hardcoded per contract) ----
N_CORES = 8
N_FULL = 32           # batch
CIN = 128             # input channels
COUT = 256            # output channels
H = W = 56
KH = KW = 3
BN_EPS = 1e-5

IMGS = N_FULL // N_CORES          # 4 images per core
WP = W + 2                        # 58 padded width
HP = H + 2
PADPIX = HP * WP                  # 3364
PIX = H * W                       # 3136
NCHUNK = COUT // 128              # 2 chunks of 128 output channels
RTR = 8                           # rows per matmul tile
RT = H // RTR                     # 7 row tiles per image
NTILE = RTR * W                   # 448 = matmul free dim (<=512, one PSUM bank)
NTOT = N_FULL * PIX               # 200704 elements per channel for BN stats
NPAIR = 14                        # DoubleRow matmuls per PSUM tile
HFX = PIX // 2                    # half-image columns (sign/DMA granularity)
WSLOT = 2 * 128                   # fp8 bytes per weight pair
PHA = 9 * WSLOT                   # phase-1 weight bytes (pairs 0-8)
PHB = 5 * WSLOT                   # phase-2 weight bytes (pairs 9-13)

BF16 = mybir.dt.bfloat16
F32 = mybir.dt.float32
FP8 = mybir.dt.float8e4
NP8 = ml_dtypes.float8_e4m3

_MAX_DRAIN_WAITS = 1  # walrus CTRL instructions accept a single sync wait


def _split_multi_waits(nc, max_waits=1):
    """This walrus build rejects instructions with more than one sem wait.
    Hoist excess waits onto same-engine NoOps inserted immediately before the
    offending instruction (the engine blocks at the NoOp instead — identical
    ordering semantics)."""
    ctr = 0
    for bbw in nc.main_func.blocks:
        out = []
        changed = False
        for inst in bbw.instructions:
            si = inst.sync_info
            w = list(si.on_wait or []) if si else []
            if len(w) > max_waits:
                changed = True
                excess = w[: len(w) - max_waits]
                for i in range(0, len(excess), max_waits):
                    nop = mybir.InstNoOp(name=f"WFIX-{ctr}", ins=[], outs=[])
                    ctr += 1
                    nop.engine = inst.engine
                    nop.sync_info = mybir.SyncInfo(
                        on_wait=excess[i : i + max_waits], on_update=[]
                    )
                    out.append(nop)
                inst.sync_info = mybir.SyncInfo(
                    on_wait=w[len(w) - max_waits :],
                    on_update=list(si.on_update or []),
                )
            out.append(inst)
        if changed:
            bbw.instructions = out
    return ctr


class _SplitDrainTileContext(tile.TileContext):
    """TileContext whose final drain splits its sem waits across multiple
    sync-engine instructions (this walrus build caps CTRL waits at 1)."""

    def _drain_and_barrier(self, tick_clock, wait_clock):
        drain_inst = self.nc.sync.drain()
        wait_clock.add_sem_waits(
            drain_inst.ins, ScopedClock({None: tick_clock.global_clock})
        )
        si = drain_inst.ins.sync_info
        w = list(si.on_wait or [])
        if len(w) > _MAX_DRAIN_WAITS:
            drain_inst.ins.sync_info = mybir.SyncInfo(
                on_wait=w[:_MAX_DRAIN_WAITS], on_update=list(si.on_update or [])
            )
            for i in range(_MAX_DRAIN_WAITS, len(w), _MAX_DRAIN_WAITS):
                nop = self.nc.sync.nop(nofuse=True)
                nop.ins.sync_info = mybir.SyncInfo(
                    on_wait=w[i : i + _MAX_DRAIN_WAITS], on_update=[]
                )
        self.nc.all_engine_barrier()
        assert self.sems is not None
        popped = self.nc._tile_sem_poison_stack.pop()
        assert popped is self._sem_poison
        self.nc.clear_and_free_semaphores(list(self.sems.allocated().values()))
        self.nc.all_engine_barrier()


def _pair_rhs(xa_v, xb_v, rt, pr):
    """rhs AP [128, 2, 8, 56] for DoubleRow pair pr of row-tile rt.

    xa_v: [128, 2, HP, WP] view of planes (q0, q1);
    xb_v: [128, 3, HP, WP] view of planes (q2, q2<<1, q2<<59)."""
    r0 = rt * RTR
    if pr < 9:                       # (tap pr, q0) + (tap pr, q1)
        dy, dx = divmod(pr, KW)
        return xa_v[:, 0:2, r0 + dy : r0 + dy + RTR, dx : dx + W]
    if pr < 12:                      # q2 taps (dy,0)+(dy,1) via shift-1 plane
        dy = pr - 9
        return xb_v[:, 0:2, r0 + dy : r0 + dy + RTR, 0:W]
    if pr == 12:                     # q2 taps (0,2)+(1,2) via shift-59 plane
        return xb_v[:, 1:3, r0 : r0 + RTR, 1 : 1 + W]
    # pr == 13: q2 tap (2,2), second half has zero weights
    return xb_v[:, 0:1, r0 + 2 : r0 + 2 + RTR, 2 : 2 + W].broadcast_to(
        [128, 2, RTR, W]
    )


def build_bass(n_cores=N_CORES, collective=True):
    """Build the per-core Bass module (SPMD: same program on every core)."""
    nc = bass.Bass(num_devices=n_cores)

    xq_d = nc.dram_tensor("xq", [IMGS, CIN, 5 * PADPIX], FP8,
                          kind="ExternalInput")
    wq_d = nc.dram_tensor("wq", [CIN, NCHUNK * NPAIR * WSLOT], FP8,
                          kind="ExternalInput")
    # pqrb[p, j-col chunks]: P | Qc | R | beta  (4 cols per chunk)
    pqrb_d = nc.dram_tensor("pqrb", [128, 4 * NCHUNK], F32,
                            kind="ExternalInput")
    out_d = nc.dram_tensor("out", [IMGS, NCHUNK, 128, PIX], FP8,
                           kind="ExternalOutput")

    with _SplitDrainTileContext(nc) as tc:
        with (
            tc.tile_pool(name="const", bufs=1) as constp,
            tc.tile_pool(name="xbuf", bufs=1) as xp,
            tc.tile_pool(name="zbuf", bufs=1) as zp,
            tc.tile_pool(name="stats", bufs=1) as sp,
            tc.tile_pool(name="sq", bufs=2) as sqp,
            tc.tile_pool(name="ost", bufs=6) as op_,
            tc.tile_pool(name="pz", bufs=8, space="PSUM") as pp,
            tc.tile_pool(name="dram", bufs=1, space="DRAM") as dp,
        ):
            # ---- weights (split per chunk/phase for early PE start) ----
            wa = [constp.tile([128, PHA], FP8, tag=f"wa{j}", name=f"wa{j}")
                  for j in range(NCHUNK)]
            wb = [constp.tile([128, PHB], FP8, tag=f"wb{j}", name=f"wb{j}")
                  for j in range(NCHUNK)]
            pqrb_sb = constp.tile([128, 4 * NCHUNK], F32, tag="pqrb")

            # ---- x plane stacks (per image, split planes 01 / 234) ----
            xa = [xp.tile([128, 2 * PADPIX], FP8, tag=f"xa{i}", name=f"xa{i}")
                  for i in range(IMGS)]
            xb = [xp.tile([128, 3 * PADPIX], FP8, tag=f"xb{i}", name=f"xb{i}")
                  for i in range(IMGS)]

            # DMA issue order = arrival order: w0a, img0 planes, w0b, then
            # the rest.  First matmul waits only on w0a + xa[0].
            nc.sync.dma_start(wa[0][:], wq_d[:, 0:PHA])
            nc.sync.dma_start(xa[0][:], xq_d[0][:, 0 : 2 * PADPIX])
            nc.sync.dma_start(xb[0][:], xq_d[0][:, 2 * PADPIX :])
            nc.sync.dma_start(wb[0][:], wq_d[:, PHA : PHA + PHB])
            nc.sync.dma_start(pqrb_sb[:], pqrb_d[:])
            for i in range(1, IMGS):
                nc.sync.dma_start(xa[i][:], xq_d[i][:, 0 : 2 * PADPIX])
                nc.sync.dma_start(xb[i][:], xq_d[i][:, 2 * PADPIX :])
            off = NPAIR * WSLOT
            nc.sync.dma_start(wa[1][:], wq_d[:, off : off + PHA])
            nc.sync.dma_start(wb[1][:], wq_d[:, off + PHA : off + PHA + PHB])

            # ---- z buffers + stats ----
            z = [zp.tile([128, IMGS * PIX], F32, tag=f"z{j}", name=f"z{j}")
                 for j in range(NCHUNK)]
            ssum = sp.tile([128, 64], F32, tag="ssum")
            ssq = sp.tile([128, 64], F32, tag="ssq")

            P_ = pqrb_sb[:, 0:NCHUNK]
            Qc = pqrb_sb[:, NCHUNK : 2 * NCHUNK]
            R_ = pqrb_sb[:, 2 * NCHUNK : 3 * NCHUNK]
            beta = pqrb_sb[:, 3 * NCHUNK : 4 * NCHUNK]
            inv_n = 1.0 / NTOT
            npart = IMGS * RT

            def _consume(j, img, rt, pt):
                """DVE stats consumers for one finished PSUM tile: z-copy
                with sum accumulation, then square with sumsq accumulation."""
                col = img * RT + rt
                zs = z[j][:, img * PIX + rt * NTILE
                          : img * PIX + (rt + 1) * NTILE]
                nc.vector.tensor_scalar(
                    out=zs, in0=pt[:], scalar1=0.0, scalar2=None,
                    op0=mybir.AluOpType.add, op1=mybir.AluOpType.add,
                    accum_out=ssum[:, j * npart + col : j * npart + col + 1],
                )
                sqt = sqp.tile([128, NTILE], F32, tag="sqt")
                nc.vector.scalar_tensor_tensor(
                    out=sqt[:], in0=pt[:], scalar=1.0, in1=zs,
                    op0=mybir.AluOpType.mult, op1=mybir.AluOpType.mult,
                    accum_out=ssq[:, j * npart + col : j * npart + col + 1],
                )

            def _mm(pt, wv, xa_v, xb_v, rt, pr):
                nc.tensor.matmul(
                    pt[:], wv[:, pr] if pr < 9 else wv[:, pr - 9],
                    _pair_rhs(xa_v, xb_v, rt, pr),
                    start=(pr == 0), stop=(pr == 12),
                    perf_mode=mybir.MatmulPerfMode.DoubleRow,
                )

            TILE_ORDER = list(range(9)) + [9, 10, 11, 13, 12]

            for j in range(NCHUNK):
                wa_v = wa[j][:].rearrange("p (pr k o) -> p pr k o", pr=9, k=2)
                wb_v = wb[j][:].rearrange("p (pr k o) -> p pr k o", pr=5, k=2)

                for img in range(IMGS):
                    xa_v = xa[img][:].rearrange("p (t r c) -> p t r c",
                                                t=2, r=HP)
                    xb_v = xb[img][:].rearrange("p (t r c) -> p t r c",
                                                t=3, r=HP)
                    pts = [pp.tile([128, NTILE], F32, tag="pz",
                                   name=f"pz{j}_{img}_{rt}")
                           for rt in range(RT)]
                    if j == 0 and img == 0:
                        # Phased: pairs 0-8 (planes q0/q1, early DMA) across
                        # all row tiles first — a 63-matmul runway while
                        # xb[0] is still in flight.
                        for rt in range(RT):
                            for pr in range(9):
                                _mm(pts[rt], wa_v, xa_v, xb_v, rt, pr)
                        for rt in range(RT):
                            for pr in (9, 10, 11, 13, 12):
                                _mm(pts[rt], wb_v, xa_v, xb_v, rt, pr)
                            _consume(j, img, rt, pts[rt])
                    else:
                        # Tile-major: each tile's 14 pairs are consecutive so
                        # group-closures are spaced a full tile apart and the
                        # DVE consumers (~1184 ns/tile < 1306 ns/tile) keep up.
                        for rt in range(RT):
                            for pr in TILE_ORDER:
                                _mm(pts[rt], wa_v if pr < 9 else wb_v,
                                    xa_v, xb_v, rt, pr)
                            _consume(j, img, rt, pts[rt])

                # ---- chunk-j stats on Pool: [128,2] = (sum, sumsq) ----
                cc_sb = sp.tile([128, 2], F32, tag=f"ccsb{j}", name=f"ccsb{j}")
                nc.vector.reduce_sum(
                    out=cc_sb[:, 0:1], in_=ssum[:, j * npart : (j + 1) * npart],
                    axis=mybir.AxisListType.X,
                )
                nc.vector.reduce_sum(
                    out=cc_sb[:, 1:2], in_=ssq[:, j * npart : (j + 1) * npart],
                    axis=mybir.AxisListType.X,
                )
                st = sp.tile([128, 2], F32, tag=f"st{j}", name=f"st{j}")
                if collective and n_cores > 1:
                    cc_in = dp.tile([128, 2], F32, tag=f"ccin{j}",
                                    name=f"ccin{j}")
                    cc_out = dp.tile([128, 2], F32, tag=f"ccout{j}",
                                     name=f"ccout{j}")
                    nc.sync.dma_start(cc_in[:], cc_sb[:])
                    nc.gpsimd.collective_compute(
                        "AllReduce", mybir.AluOpType.add,
                        replica_groups=[list(range(n_cores))],
                        ins=[cc_in.opt()], outs=[cc_out.opt()],
                    )
                    nc.sync.dma_start(st[:], cc_out[:])
                else:
                    nc.gpsimd.tensor_copy(st[:], cc_sb[:])

                # ---- BN affine:  A = R*rsqrt(P*q - Qc*s^2 + eps) >= 0,
                #      B = beta - mu*A,  T = mu - beta/A  (tail chunk only).
                # Chunk 0's math runs on Pool (DVE is busy with tile
                # consumers of chunk 1); the tail chunk's runs on DVE.
                last = j == NCHUNK - 1
                eng = nc.vector if last else nc.gpsimd
                Pj, Qj, Rj, bj = (v[:, j : j + 1] for v in (P_, Qc, R_, beta))
                s0, s1 = st[:, 0:1], st[:, 1:2]
                mu = sp.tile([128, 1], F32, tag=f"mu{j}", name=f"mu{j}")
                u = sp.tile([128, 1], F32, tag=f"u{j}", name=f"u{j}")
                A = sp.tile([128, 1], F32, tag=f"A{j}", name=f"A{j}")
                B = sp.tile([128, 1], F32, tag=f"B{j}", name=f"B{j}")
                t1 = sp.tile([128, 1], F32, tag=f"t1{j}", name=f"t1{j}")

                eng.tensor_tensor(out=u[:], in0=s1, in1=Pj,
                                  op=mybir.AluOpType.mult)
                eng.tensor_tensor(out=t1[:], in0=s0, in1=s0,
                                  op=mybir.AluOpType.mult)
                eng.tensor_tensor(out=t1[:], in0=t1[:], in1=Qj,
                                  op=mybir.AluOpType.mult)
                eng.tensor_tensor(out=u[:], in0=u[:], in1=t1[:],
                                  op=mybir.AluOpType.subtract)
                eng.tensor_scalar(out=u[:], in0=u[:], scalar1=float(BN_EPS),
                                  scalar2=None, op0=mybir.AluOpType.add)
                nc.vector.reciprocal(u[:], u[:])
                nc.scalar.activation(out=u[:], in_=u[:],
                                     func=mybir.ActivationFunctionType.Sqrt)
                eng.tensor_tensor(out=A[:], in0=Rj, in1=u[:],
                                  op=mybir.AluOpType.mult)
                eng.tensor_scalar(out=mu[:], in0=s0, scalar1=inv_n,
                                  scalar2=None, op0=mybir.AluOpType.mult)
                eng.tensor_tensor(out=t1[:], in0=mu[:], in1=A[:],
                                  op=mybir.AluOpType.mult)
                eng.tensor_tensor(out=B[:], in0=bj, in1=t1[:],
                                  op=mybir.AluOpType.subtract)

                def _act_sign(img, h, tag):
                    """ACT Sign on half-image h of img -> staged -> DRAM."""
                    o = op_.tile([128, HFX], FP8, tag="ostg", name=tag)
                    lo = img * PIX + h * HFX
                    nc.scalar.activation(
                        out=o[:], in_=z[j][:, lo : lo + HFX],
                        func=mybir.ActivationFunctionType.Sign,
                        bias=B[:, 0:1], scale=A[:, 0:1],
                    )
                    nc.sync.dma_start(
                        out_d[img, j][:, h * HFX : (h + 1) * HFX], o[:])

                def _act_sign_t(img, h):
                    _act_sign(img, h, f"ostg{j}_{img}_{h}")

                if not last:
                    # ---- chunk 0: ACT signs all 4 images as 8 half-image
                    # passes (overlaps chunk-1 conv; ACT has no other work) --
                    for img in range(IMGS):
                        for h in range(2):
                            _act_sign(img, h, f"ostg{j}_{img}_{h}")
                    continue

                # ---- tail chunk: ACT takes imgs 0-1, DVE takes imgs 2-3
                # as single-pass compares (z >= T) -> {1,0}; the host maps
                # {1,0} -> {+1,-1} for exactly these slices. ----
                T = sp.tile([128, 1], F32, tag="T", name="T")
                nc.vector.reciprocal(t1[:], A[:])
                nc.vector.tensor_tensor(out=t1[:], in0=bj, in1=t1[:],
                                        op=mybir.AluOpType.mult)
                nc.vector.tensor_tensor(out=T[:], in0=mu[:], in1=t1[:],
                                        op=mybir.AluOpType.subtract)

                def _dve_cmp(img, h):
                    o = op_.tile([128, HFX], FP8, tag="ostg",
                                 name=f"ostg{j}_{img}_{h}")
                    lo = img * PIX + h * HFX
                    nc.vector.tensor_tensor(
                        out=o[:], in0=z[j][:, lo : lo + HFX],
                        in1=T[:, 0:1].broadcast_to([128, HFX]),
                        op=mybir.AluOpType.is_ge,
                    )
                    nc.sync.dma_start(
                        out_d[img, j][:, h * HFX : (h + 1) * HFX], o[:])

                # interleaved by expected completion so the in-order SP DMA
                # queue never head-of-line blocks on a not-yet-ready half
                for (eng_f, img, h) in (
                    (_act_sign_t, 0, 0), (_dve_cmp, 2, 0),
                    (_act_sign_t, 0, 1), (_dve_cmp, 2, 1),
                    (_act_sign_t, 1, 0), (_dve_cmp, 3, 0),
                    (_act_sign_t, 1, 1), (_dve_cmp, 3, 1),
                ):
                    eng_f(img, h)

    _split_multi_waits(nc)
    return nc


def _prep_inputs(x, weight, gamma, beta):
    """Host-side prep: sign/alpha/gamma folding, padding, 3-term fp8 split."""
    x = np.ascontiguousarray(x, dtype=np.float32)
    weight = np.ascontiguousarray(weight, dtype=np.float32)
    gamma = np.asarray(gamma, np.float32)
    beta = np.asarray(beta, np.float32)

    alpha = np.abs(weight).mean(axis=(1, 2, 3)).astype(np.float32)      # [256]
    S = np.where(gamma >= 0, np.float32(1), np.float32(-1))
    sgn = np.where(weight >= 0, np.float32(1), np.float32(-1)) * S[:, None, None, None]

    # ---- fp8 weight pairs: wq[cin, j, pair, ktile, o] ----
    sgn_t = sgn.transpose(1, 2, 3, 0).reshape(CIN, KH * KW, NCHUNK, 128)
    wq = np.zeros((CIN, NCHUNK, NPAIR, 2, 128), np.float32)
    S1, S2 = 2.0 ** -4, 2.0 ** -6
    for k in range(9):                      # pairs 0-8: (q0, q1) of tap k
        wq[:, :, k, 0] = sgn_t[:, k]
        wq[:, :, k, 1] = sgn_t[:, k] * S1
    for dy in range(3):                     # pairs 9-11: q2 taps (dy,0)+(dy,1)
        wq[:, :, 9 + dy, 0] = sgn_t[:, dy * 3 + 0] * S2
        wq[:, :, 9 + dy, 1] = sgn_t[:, dy * 3 + 1] * S2
    wq[:, :, 12, 0] = sgn_t[:, 2] * S2      # pair 12: q2 taps (0,2)+(1,2)
    wq[:, :, 12, 1] = sgn_t[:, 5] * S2
    wq[:, :, 13, 0] = sgn_t[:, 8] * S2      # pair 13: q2 tap (2,2) + zeros
    wq = np.ascontiguousarray(
        wq.reshape(CIN, NCHUNK * NPAIR * WSLOT)
    ).astype(NP8)

    # pqrb[p, j]: P = a^2/N | Qc = a^2/N^2 | R = a*|g| | beta
    def chunked(v):
        return np.ascontiguousarray(v.reshape(NCHUNK, 128).T)  # [128, 2]
    a2 = alpha * alpha
    pqrb = np.concatenate(
        [chunked(a2 / NTOT), chunked(a2 / NTOT / NTOT),
         chunked(alpha * np.abs(gamma)), chunked(beta)], axis=1
    ).astype(np.float32)                                                # [128, 8]

    # ---- 3-term fp8 split of padded x, with shifted q2 planes ----
    xpad = np.zeros((N_FULL, CIN, HP, WP), np.float32)
    xpad[:, :, 1 : H + 1, 1 : W + 1] = x
    xpad = xpad.reshape(N_FULL, CIN, PADPIX)
    q0 = xpad.astype(NP8)
    r1 = xpad - q0.astype(np.float32)
    q1 = (r1 * 16.0).astype(NP8)
    r2 = r1 - q1.astype(np.float32) * (1.0 / 16.0)
    q2 = (r2 * 64.0).astype(NP8)
    q2p = np.zeros((N_FULL, CIN, PADPIX + 64), NP8)
    q2p[:, :, :PADPIX] = q2
    xq = np.stack(
        [q0, q1, q2, q2p[:, :, 1 : 1 + PADPIX], q2p[:, :, 59 : 59 + PADPIX]],
        axis=2,
    )                                                   # [N, CIN, 5, PADPIX]
    xq = np.ascontiguousarray(xq.reshape(N_FULL, CIN, 5 * PADPIX))

    in_maps = []
    for c in range(N_CORES):
        sl = slice(c * IMGS, (c + 1) * IMGS)
        in_maps.append({
            "xq": np.ascontiguousarray(xq[sl]),
            "wq": wq,
            "pqrb": pqrb,
        })
    return in_maps


def kernel(x, weight, gamma, beta):
    in_maps = _prep_inputs(x, weight, gamma, beta)
    nc = build_bass()
    res = run_bass_kernel_spmd(nc, in_maps, core_ids=list(range(N_CORES)))
    out = np.empty((N_FULL, COUT, H, W), np.float32)
    for c in range(N_CORES):
        o = res.results[c]["out"].astype(np.float32)   # [IMGS,2,128,PIX] fp8
        o[2:4, 1] = o[2:4, 1] * 2.0 - 1.0   # DVE compare slices: {1,0}->{+-1}
        out[c * IMGS : (c + 1) * IMGS] = o.reshape(IMGS, COUT, H, W)
    return out


# revision 15
# speedup vs baseline: 2.3357x; 1.0207x over previous
"""Trainium2 Bass kernel for ConvBnSign (binarized 3x3 conv + sync-BN + sign).

Math: y = conv2d(x, sign(w) * alpha)  with alpha = mean|w| per out-channel,
then train-mode BatchNorm over (N,H,W), then hard_sign.

Folds: alpha > 0 folds into the BN affine; S = sign(gamma) folds into the
binarized weights (z' = S*z), making the BN scale A = alpha*|gamma|*rsqrt(
alpha^2 var + eps) >= 0 so the final sign is also a per-channel threshold
compare  out = (z' >= T) ? +1 : -1,  T = mu' - beta/A  — which lets the
tail's sign pass split across ACT / DVE / Pool engines.

Precision: x is split on host into 3 fp8-e4m3 terms
  x ~ q0 + q1/16 + q2/64   (residual rms ~1.6e-5 relative),
with per-term scales folded into the fp8 weights (1, 2^-4, 2^-6 — all
normal in e4m3). Each conv tile is 14 DoubleRow fp8 matmuls (2 k-tiles
each, 0.5 cyc/row): the 27 (tap, term) k-tiles pair via the plane dim of
the SBUF layout, where two extra planes are host-shifted copies of q2
(shift +1 / +59) so cross-tap pairs land at the uniform plane stride.

Per-tile stats consumers both run on DVE (tensor_scalar z-copy+sum and
tensor_tensor_reduce square+sum), keeping ACT free for Sign and Pool free
for reduces/BN math — avoids ACT head-of-line blocking on PSUM drains.

Sharding: data-parallel, 4 images per core across 8 cores; BN stats are
per-channel partial sums [128,4] fp32 all-reduced across cores.
"""

import numpy as np
import ml_dtypes

import concourse.bass as bass
import concourse.mybir as mybir
import concourse.tile as tile
from concourse.vector_clock import ScopedClock
from concourse.bass_utils import run_bass_kernel_spmd

# ---- problem constants (hardcoded per contract) ----
N_CORES = 8
N_FULL = 32           # batch
CIN = 128             # input channels
COUT = 256            # output channels
H = W = 56
KH = KW = 3
BN_EPS = 1e-5

IMGS = N_FULL // N_CORES          # 4 images per core
WP = W + 2                        # 58 padded width
HP = H + 2
PADPIX = HP * WP                  # 3364
PIX = H * W                       # 3136
NCHUNK = COUT // 128              # 2 chunks of 128 output channels
RTR = 8                           # rows per matmul tile
RT = H // RTR                     # 7 row tiles per image
NTILE = RTR * W                   # 448 = matmul free dim (<=512, one PSUM bank)
NTOT = N_FULL * PIX               # 200704 elements per channel for BN stats
NPAIR = 14                        # DoubleRow matmuls per PSUM tile
HFX = PIX // 2                    # half-image columns (sign/DMA granularity)
WSLOT = 2 * 128                   # fp8 bytes per weight pair
PHA = 9 * WSLOT                   # phase-1 weight bytes (pairs 0-8)
PHB = 5 * WSLOT                   # phase-2 weight bytes (pairs 9-13)

BF16 = mybir.dt.bfloat16
F32 = mybir.dt.float32
FP8 = mybir.dt.float8e4
NP8 = ml_dtypes.float8_e4m3

_MAX_DRAIN_WAITS = 1  # walrus CTRL instructions accept a single sync wait


def _split_multi_waits(nc, max_waits=1):
    """This walrus build rejects instructions with more than one sem wait.
    Hoist excess waits onto same-engine NoOps inserted immediately before the
    offending instruction (the engine blocks at the NoOp instead — identical
    ordering semantics)."""
    ctr = 0
    for bbw in nc.main_func.blocks:
        out = []
        changed = False
        for inst in bbw.instructions:
            si = inst.sync_info
            w = list(si.on_wait or []) if si else []
            if len(w) > max_waits:
                changed = True
                excess = w[: len(w) - max_waits]
                for i in range(0, len(excess), max_waits):
                    nop = mybir.InstNoOp(name=f"WFIX-{ctr}", ins=[], outs=[])
                    ctr += 1
                    nop.engine = inst.engine
                    nop.sync_info = mybir.SyncInfo(
                        on_wait=excess[i : i + max_waits], on_update=[]
                    )
                    out.append(nop)
                inst.sync_info = mybir.SyncInfo(
                    on_wait=w[len(w) - max_waits :],
                    on_update=list(si.on_update or []),
                )
            out.append(inst)
        if changed:
            bbw.instructions = out
    return ctr


class _SplitDrainTileContext(tile.TileContext):
    """TileContext whose final drain splits its sem waits across multiple
    sync-engine instructions (this walrus build caps CTRL waits at 1)."""

    def _drain_and_barrier(self, tick_clock, wait_clock):
        drain_inst = self.nc.sync.drain()
        wait_clock.add_sem_waits(
            drain_inst.ins, ScopedClock({None: tick_clock.global_clock})
        )
        si = drain_inst.ins.sync_info
        w = list(si.on_wait or [])
        if len(w) > _MAX_DRAIN_WAITS:
            drain_inst.ins.sync_info = mybir.SyncInfo(
                on_wait=w[:_MAX_DRAIN_WAITS], on_update=list(si.on_update or [])
            )
            for i in range(_MAX_DRAIN_WAITS, len(w), _MAX_DRAIN_WAITS):
                nop = self.nc.sync.nop(nofuse=True)
                nop.ins.sync_info = mybir.SyncInfo(
                    on_wait=w[i : i + _MAX_DRAIN_WAITS], on_update=[]
                )
        self.nc.all_engine_barrier()
        assert self.sems is not None
        popped = self.nc._tile_sem_poison_stack.pop()
        assert popped is self._sem_poison
        self.nc.clear_and_free_semaphores(list(self.sems.allocated().values()))
        self.nc.all_engine_barrier()


def _pair_rhs(xa_v, xb_v, rt, pr):
    """rhs AP [128, 2, 8, 56] for DoubleRow pair pr of row-tile rt.

    xa_v: [128, 2, HP, WP] view of planes (q0, q1);
    xb_v: [128, 3, HP, WP] view of planes (q2, q2<<1, q2<<59)."""
    r0 = rt * RTR
    if pr < 9:                       # (tap pr, q0) + (tap pr, q1)
        dy, dx = divmod(pr, KW)
        return xa_v[:, 0:2, r0 + dy : r0 + dy + RTR, dx : dx + W]
    if pr < 12:                      # q2 taps (dy,0)+(dy,1) via shift-1 plane
        dy = pr - 9
        return xb_v[:, 0:2, r0 + dy : r0 + dy + RTR, 0:W]
    if pr == 12:                     # q2 taps (0,2)+(1,2) via shift-59 plane
        return xb_v[:, 1:3, r0 : r0 + RTR, 1 : 1 + W]
    # pr == 13: q2 tap (2,2), second half has zero weights
    return xb_v[:, 0:1, r0 + 2 : r0 + 2 + RTR, 2 : 2 + W].broadcast_to(
        [128, 2, RTR, W]
    )


def build_bass(n_cores=N_CORES, collective=True):
    """Build the per-core Bass module (SPMD: same program on every core)."""
    nc = bass.Bass(num_devices=n_cores)

    xq_d = nc.dram_tensor("xq", [IMGS, CIN, 5 * PADPIX], FP8,
                          kind="ExternalInput")
    wq_d = nc.dram_tensor("wq", [CIN, NCHUNK * NPAIR * WSLOT], FP8,
                          kind="ExternalInput")
    # pqrb[p, j-col chunks]: P | Qc | R | beta  (4 cols per chunk)
    pqrb_d = nc.dram_tensor("pqrb", [128, 4 * NCHUNK], F32,
                            kind="ExternalInput")
    out_d = nc.dram_tensor("out", [IMGS, NCHUNK, 128, PIX], FP8,
                           kind="ExternalOutput")

    with _SplitDrainTileContext(nc) as tc:
        with (
            tc.tile_pool(name="const", bufs=1) as constp,
            tc.tile_pool(name="xbuf", bufs=1) as xp,
            tc.tile_pool(name="zbuf", bufs=1) as zp,
            tc.tile_pool(name="stats", bufs=1) as sp,
            tc.tile_pool(name="sq", bufs=2) as sqp,
            tc.tile_pool(name="ost", bufs=6) as op_,
            tc.tile_pool(name="pz", bufs=8, space="PSUM") as pp,
            tc.tile_pool(name="dram", bufs=1, space="DRAM") as dp,
        ):
            # ---- weights (split per chunk/phase for early PE start) ----
            wa = [constp.tile([128, PHA], FP8, tag=f"wa{j}", name=f"wa{j}")
                  for j in range(NCHUNK)]
            wb = [constp.tile([128, PHB], FP8, tag=f"wb{j}", name=f"wb{j}")
                  for j in range(NCHUNK)]
            pqrb_sb = constp.tile([128, 4 * NCHUNK], F32, tag="pqrb")

            # ---- x plane stacks (per image, split planes 01 / 234) ----
            xa = [xp.tile([128, 2 * PADPIX], FP8, tag=f"xa{i}", name=f"xa{i}")
                  for i in range(IMGS)]
            xb = [xp.tile([128, 3 * PADPIX], FP8, tag=f"xb{i}", name=f"xb{i}")
                  for i in range(IMGS)]

            # DMA issue order = arrival order: w0a, img0 planes, w0b, then
            # the rest.  First matmul waits only on w0a + xa[0].
            nc.sync.dma_start(wa[0][:], wq_d[:, 0:PHA])
            nc.sync.dma_start(xa[0][:], xq_d[0][:, 0 : 2 * PADPIX])
            nc.sync.dma_start(xb[0][:], xq_d[0][:, 2 * PADPIX :])
            nc.sync.dma_start(wb[0][:], wq_d[:, PHA : PHA + PHB])
            nc.sync.dma_start(pqrb_sb[:], pqrb_d[:])
            for i in range(1, IMGS):
                nc.sync.dma_start(xa[i][:], xq_d[i][:, 0 : 2 * PADPIX])
                nc.sync.dma_start(xb[i][:], xq_d[i][:, 2 * PADPIX :])
            off = NPAIR * WSLOT
            nc.sync.dma_start(wa[1][:], wq_d[:, off : off + PHA])
            nc.sync.dma_start(wb[1][:], wq_d[:, off + PHA : off + PHA + PHB])

            # ---- PE warmup: dummy matmuls ramp the tensor-engine clock
            # to full speed while the x/w DMAs are in flight ----
            wu = constp.tile([128, 520], FP8, tag="warm")
            wscr = constp.tile([128, 1], F32, tag="wscr")
            nc.gpsimd.memset(wu[:], 0)
            pw = pp.tile([128, 512], F32, tag="pz", name="warm_ps")
            for i in range(7):
                nc.tensor.matmul(pw[0:1, :], wu[:, i : i + 1], wu[:, 8:520],
                                 start=True, stop=True)
            nc.vector.tensor_copy(wscr[0:1, 0:1], pw[0:1, 0:1])

            # ---- z buffers + stats ----
            z = [zp.tile([128, IMGS * PIX], F32, tag=f"z{j}", name=f"z{j}")
                 for j in range(NCHUNK)]
            ssum = sp.tile([128, 64], F32, tag="ssum")

            ssq = sp.tile([128, 64], F32, tag="ssq")

            P_ = pqrb_sb[:, 0:NCHUNK]
            Qc = pqrb_sb[:, NCHUNK : 2 * NCHUNK]
            R_ = pqrb_sb[:, 2 * NCHUNK : 3 * NCHUNK]
            beta = pqrb_sb[:, 3 * NCHUNK : 4 * NCHUNK]
            inv_n = 1.0 / NTOT
            npart = IMGS * RT

            def _consume(j, img, rt, pt):
                """DVE stats consumers for one finished PSUM tile: z-copy
                with sum accumulation, then square with sumsq accumulation."""
                col = img * RT + rt
                zs = z[j][:, img * PIX + rt * NTILE
                          : img * PIX + (rt + 1) * NTILE]
                nc.vector.tensor_scalar(
                    out=zs, in0=pt[:], scalar1=0.0, scalar2=None,
                    op0=mybir.AluOpType.add, op1=mybir.AluOpType.add,
                    accum_out=ssum[:, j * npart + col : j * npart + col + 1],
                )
                sqt = sqp.tile([128, NTILE], F32, tag="sqt")
                nc.vector.scalar_tensor_tensor(
                    out=sqt[:], in0=pt[:], scalar=1.0, in1=zs,
                    op0=mybir.AluOpType.mult, op1=mybir.AluOpType.mult,
                    accum_out=ssq[:, j * npart + col : j * npart + col + 1],
                )

            def _mm(pt, wv, xa_v, xb_v, rt, pr):
                nc.tensor.matmul(
                    pt[:], wv[:, pr] if pr < 9 else wv[:, pr - 9],
                    _pair_rhs(xa_v, xb_v, rt, pr),
                    start=(pr == 0), stop=(pr == 12),
                    perf_mode=mybir.MatmulPerfMode.DoubleRow,
                )

            TILE_ORDER = list(range(9)) + [9, 10, 11, 13, 12]

            for j in range(NCHUNK):
                wa_v = wa[j][:].rearrange("p (pr k o) -> p pr k o", pr=9, k=2)
                wb_v = wb[j][:].rearrange("p (pr k o) -> p pr k o", pr=5, k=2)

                for img in range(IMGS):
                    xa_v = xa[img][:].rearrange("p (t r c) -> p t r c",
                                                t=2, r=HP)
                    xb_v = xb[img][:].rearrange("p (t r c) -> p t r c",
                                                t=3, r=HP)
                    pts = [pp.tile([128, NTILE], F32, tag="pz",
                                   name=f"pz{j}_{img}_{rt}")
                           for rt in range(RT)]
                    if j == 0 and img == 0:
                        # Phased: pairs 0-8 (planes q0/q1, early DMA) across
                        # all row tiles first — a 63-matmul runway while
                        # xb[0] is still in flight.
                        for rt in range(RT):
                            for pr in range(9):
                                _mm(pts[rt], wa_v, xa_v, xb_v, rt, pr)
                        for rt in range(RT):
                            for pr in (9, 10, 11, 13, 12):
                                _mm(pts[rt], wb_v, xa_v, xb_v, rt, pr)
                            _consume(j, img, rt, pts[rt])
                    else:
                        # Tile-major: each tile's 14 pairs are consecutive so
                        # group-closures are spaced a full tile apart and the
                        # DVE consumers (~1184 ns/tile < 1306 ns/tile) keep up.
                        for rt in range(RT):
                            for pr in TILE_ORDER:
                                _mm(pts[rt], wa_v if pr < 9 else wb_v,
                                    xa_v, xb_v, rt, pr)
                            _consume(j, img, rt, pts[rt])

                # ---- chunk-j stats on Pool: [128,2] = (sum, sumsq) ----
                cc_sb = sp.tile([128, 2], F32, tag=f"ccsb{j}", name=f"ccsb{j}")
                nc.vector.reduce_sum(
                    out=cc_sb[:, 0:1], in_=ssum[:, j * npart : (j + 1) * npart],
                    axis=mybir.AxisListType.X,
                )
                nc.vector.reduce_sum(
                    out=cc_sb[:, 1:2], in_=ssq[:, j * npart : (j + 1) * npart],
                    axis=mybir.AxisListType.X,
                )
                st = sp.tile([128, 2], F32, tag=f"st{j}", name=f"st{j}")
                if collective and n_cores > 1:
                    cc_in = dp.tile([128, 2], F32, tag=f"ccin{j}",
                                    name=f"ccin{j}")
                    cc_out = dp.tile([128, 2], F32, tag=f"ccout{j}",
                                     name=f"ccout{j}")
                    nc.sync.dma_start(cc_in[:], cc_sb[:])
                    nc.gpsimd.collective_compute(
                        "AllReduce", mybir.AluOpType.add,
                        replica_groups=[list(range(n_cores))],
                        ins=[cc_in.opt()], outs=[cc_out.opt()],
                    )
                    nc.sync.dma_start(st[:], cc_out[:])
                else:
                    nc.gpsimd.tensor_copy(st[:], cc_sb[:])

                # ---- BN affine:  A = R*rsqrt(P*q - Qc*s^2 + eps) >= 0,
                #      B = beta - mu*A,  T = mu - beta/A  (tail chunk only).
                # Chunk 0's math runs on Pool (DVE is busy with tile
                # consumers of chunk 1); the tail chunk's runs on DVE.
                last = j == NCHUNK - 1
                eng = nc.vector if last else nc.gpsimd
                Pj, Qj, Rj, bj = (v[:, j : j + 1] for v in (P_, Qc, R_, beta))
                s0, s1 = st[:, 0:1], st[:, 1:2]
                mu = sp.tile([128, 1], F32, tag=f"mu{j}", name=f"mu{j}")
                u = sp.tile([128, 1], F32, tag=f"u{j}", name=f"u{j}")
                A = sp.tile([128, 1], F32, tag=f"A{j}", name=f"A{j}")
                B = sp.tile([128, 1], F32, tag=f"B{j}", name=f"B{j}")
                t1 = sp.tile([128, 1], F32, tag=f"t1{j}", name=f"t1{j}")

                eng.tensor_tensor(out=u[:], in0=s1, in1=Pj,
                                  op=mybir.AluOpType.mult)
                eng.tensor_tensor(out=t1[:], in0=s0, in1=s0,
                                  op=mybir.AluOpType.mult)
                eng.tensor_tensor(out=t1[:], in0=t1[:], in1=Qj,
                                  op=mybir.AluOpType.mult)
                eng.tensor_tensor(out=u[:], in0=u[:], in1=t1[:],
                                  op=mybir.AluOpType.subtract)
                eng.tensor_scalar(out=u[:], in0=u[:], scalar1=float(BN_EPS),
                                  scalar2=None, op0=mybir.AluOpType.add)
                nc.vector.reciprocal(u[:], u[:])
                nc.scalar.activation(out=u[:], in_=u[:],
                                     func=mybir.ActivationFunctionType.Sqrt)
                eng.tensor_tensor(out=A[:], in0=Rj, in1=u[:],
                                  op=mybir.AluOpType.mult)
                eng.tensor_scalar(out=mu[:], in0=s0, scalar1=inv_n,
                                  scalar2=None, op0=mybir.AluOpType.mult)
                eng.tensor_tensor(out=t1[:], in0=mu[:], in1=A[:],
                                  op=mybir.AluOpType.mult)
                eng.tensor_tensor(out=B[:], in0=bj, in1=t1[:],
                                  op=mybir.AluOpType.subtract)

                def _act_sign(img, h, tag):
                    """ACT Sign on half-image h of img -> staged -> DRAM."""
                    o = op_.tile([128, HFX], FP8, tag="ostg", name=tag)
                    lo = img * PIX + h * HFX
                    nc.scalar.activation(
                        out=o[:], in_=z[j][:, lo : lo + HFX],
                        func=mybir.ActivationFunctionType.Sign,
                        bias=B[:, 0:1], scale=A[:, 0:1],
                    )
                    nc.sync.dma_start(
                        out_d[img, j][:, h * HFX : (h + 1) * HFX], o[:])

                def _act_sign_t(img, h):
                    _act_sign(img, h, f"ostg{j}_{img}_{h}")

                if not last:
                    # ---- chunk 0: ACT signs all 4 images as 8 half-image
                    # passes (overlaps chunk-1 conv; ACT has no other work) --
                    for img in range(IMGS):
                        for h in range(2):
                            _act_sign(img, h, f"ostg{j}_{img}_{h}")
                    continue

                # ---- tail chunk: ACT takes imgs 0-1, DVE takes imgs 2-3
                # as single-pass compares (z >= T) -> {1,0}; the host maps
                # {1,0} -> {+1,-1} for exactly these slices. ----
                T = sp.tile([128, 1], F32, tag="T", name="T")
                nc.vector.reciprocal(t1[:], A[:])
                nc.vector.tensor_tensor(out=t1[:], in0=bj, in1=t1[:],
                                        op=mybir.AluOpType.mult)
                nc.vector.tensor_tensor(out=T[:], in0=mu[:], in1=t1[:],
                                        op=mybir.AluOpType.subtract)

                def _dve_cmp(img, h):
                    o = op_.tile([128, HFX], FP8, tag="ostg",
                                 name=f"ostg{j}_{img}_{h}")
                    lo = img * PIX + h * HFX
                    nc.vector.tensor_tensor(
                        out=o[:], in0=z[j][:, lo : lo + HFX],
                        in1=T[:, 0:1].broadcast_to([128, HFX]),
                        op=mybir.AluOpType.is_ge,
                    )
                    nc.sync.dma_start(
                        out_d[img, j][:, h * HFX : (h + 1) * HFX], o[:])

                # interleaved by expected completion so the in-order SP DMA
                # queue never head-of-line blocks on a not-yet-ready half
                for (eng_f, img, h) in (
                    (_act_sign_t, 0, 0), (_dve_cmp, 2, 0),
                    (_act_sign_t, 0, 1), (_dve_cmp, 2, 1),
                    (_act_sign_t, 1, 0), (_dve_cmp, 3, 0),
                    (_act_sign_t, 1, 1), (_dve_cmp, 3, 1),
                ):
                    eng_f(img, h)

    _split_multi_waits(nc)
    return nc


def _prep_inputs(x, weight, gamma, beta):
    """Host-side prep: sign/alpha/gamma folding, padding, 3-term fp8 split."""
    x = np.ascontiguousarray(x, dtype=np.float32)
    weight = np.ascontiguousarray(weight, dtype=np.float32)
    gamma = np.asarray(gamma, np.float32)
    beta = np.asarray(beta, np.float32)

    alpha = np.abs(weight).mean(axis=(1, 2, 3)).astype(np.float32)      # [256]
    S = np.where(gamma >= 0, np.float32(1), np.float32(-1))
    sgn = np.where(weight >= 0, np.float32(1), np.float32(-1)) * S[:, None, None, None]

    # ---- fp8 weight pairs: wq[cin, j, pair, ktile, o] ----
    sgn_t = sgn.transpose(1, 2, 3, 0).reshape(CIN, KH * KW, NCHUNK, 128)
    wq = np.zeros((CIN, NCHUNK, NPAIR, 2, 128), np.float32)
    S1, S2 = 2.0 ** -4, 2.0 ** -6
    for k in range(9):                      # pairs 0-8: (q0, q1) of tap k
        wq[:, :, k, 0] = sgn_t[:, k]
        wq[:, :, k, 1] = sgn_t[:, k] * S1
    for dy in range(3):                     # pairs 9-11: q2 taps (dy,0)+(dy,1)
        wq[:, :, 9 + dy, 0] = sgn_t[:, dy * 3 + 0] * S2
        wq[:, :, 9 + dy, 1] = sgn_t[:, dy * 3 + 1] * S2
    wq[:, :, 12, 0] = sgn_t[:, 2] * S2      # pair 12: q2 taps (0,2)+(1,2)
    wq[:, :, 12, 1] = sgn_t[:, 5] * S2
    wq[:, :, 13, 0] = sgn_t[:, 8] * S2      # pair 13: q2 tap (2,2) + zeros
    wq = np.ascontiguousarray(
        wq.reshape(CIN, NCHUNK * NPAIR * WSLOT)
    ).astype(NP8)

    # pqrb[p, j]: P = a^2/N | Qc = a^2/N^2 | R = a*|g| | beta
    def chunked(v):
        return np.ascontiguousarray(v.reshape(NCHUNK, 128).T)  # [128, 2]
    a2 = alpha * alpha
    pqrb = np.concatenate(
        [chunked(a2 / NTOT), chunked(a2 / NTOT / NTOT),
         chunked(alpha * np.abs(gamma)), chunked(beta)], axis=1
    ).astype(np.float32)                                                # [128, 8]

    # ---- 3-term fp8 split of padded x, with shifted q2 planes ----
    xpad = np.zeros((N_FULL, CIN, HP, WP), np.float32)
    xpad[:, :, 1 : H + 1, 1 : W + 1] = x
    xpad = xpad.reshape(N_FULL, CIN, PADPIX)
    q0 = xpad.astype(NP8)
    r1 = xpad - q0.astype(np.float32)
    q1 = (r1 * 16.0).astype(NP8)
    r2 = r1 - q1.astype(np.float32) * (1.0 / 16.0)
    q2 = (r2 * 64.0).astype(NP8)
    q2p = np.zeros((N_FULL, CIN, PADPIX + 64), NP8)
    q2p[:, :, :PADPIX] = q2
    xq = np.stack(
        [q0, q1, q2, q2p[:, :, 1 : 1 + PADPIX], q2p[:, :, 59 : 59 + PADPIX]],
        axis=2,
    )                                                   # [N, CIN, 5, PADPIX]
    xq = np.ascontiguousarray(xq.reshape(N_FULL, CIN, 5 * PADPIX))

    in_maps = []
    for c in range(N_CORES):
        sl = slice(c * IMGS, (c + 1) * IMGS)
        in_maps.append({
            "xq": np.ascontiguousarray(xq[sl]),
            "wq": wq,
            "pqrb": pqrb,
        })
    return in_maps


def kernel(x, weight, gamma, beta):
    in_maps = _prep_inputs(x, weight, gamma, beta)
    nc = build_bass()
    res = run_bass_kernel_spmd(nc, in_maps, core_ids=list(range(N_CORES)))
    out = np.empty((N_FULL, COUT, H, W), np.float32)
    for c in range(N_CORES):
        o = res.results[c]["out"].astype(np.float32)   # [IMGS,2,128,PIX] fp8
        o[2:4, 1] = o[2:4, 1] * 2.0 - 1.0   # DVE compare slices: {1,0}->{+-1}
        out[c * IMGS : (c + 1) * IMGS] = o.reshape(IMGS, COUT, H, W)
    return out


# revision 19
# speedup vs baseline: 2.3898x; 1.0232x over previous
"""Trainium2 Bass kernel for ConvBnSign (binarized 3x3 conv + sync-BN + sign).

Math: y = conv2d(x, sign(w) * alpha)  with alpha = mean|w| per out-channel,
then train-mode BatchNorm over (N,H,W), then hard_sign.

Folds: alpha > 0 folds into the BN affine; S = sign(gamma) folds into the
binarized weights (z' = S*z), making the BN scale A = alpha*|gamma|*rsqrt(
alpha^2 var + eps) >= 0 so the final sign is also a per-channel threshold
compare  out = (z' >= T) ? +1 : -1,  T = mu' - beta/A  — which lets the
tail's sign pass split across ACT / DVE / Pool engines.

Precision: x is split on host into 3 fp8-e4m3 terms
  x ~ q0 + q1/16 + q2/64   (residual rms ~1.6e-5 relative),
with per-term scales folded into the fp8 weights (1, 2^-4, 2^-6 — all
normal in e4m3). Each conv tile is 14 DoubleRow fp8 matmuls (2 k-tiles
each, 0.5 cyc/row): the 27 (tap, term) k-tiles pair via the plane dim of
the SBUF layout, where two extra planes are host-shifted copies of q2
(shift +1 / +59) so cross-tap pairs land at the uniform plane stride.

Per-tile stats consumers both run on DVE (tensor_scalar z-copy+sum and
tensor_tensor_reduce square+sum), keeping ACT free for Sign and Pool free
for reduces/BN math — avoids ACT head-of-line blocking on PSUM drains.

Sharding: data-parallel, 4 images per core across 8 cores; BN stats are
per-channel partial sums [128,4] fp32 all-reduced across cores.
"""

import numpy as np
import ml_dtypes

import concourse.bass as bass
import concourse.mybir as mybir
import concourse.tile as tile
from concourse.vector_clock import ScopedClock
from concourse.bass_utils import run_bass_kernel_spmd

# ---- problem constants (hardcoded per contract) ----
N_CORES = 8
N_FULL = 32           # batch
CIN = 128             # input channels
COUT = 256            # output channels
H = W = 56
KH = KW = 3
BN_EPS = 1e-5

IMGS = N_FULL // N_CORES          # 4 images per core
WP = W + 2                        # 58 padded width
HP = H + 2
PADPIX = HP * WP                  # 3364
PIX = H * W                       # 3136
NCHUNK = COUT // 128              # 2 chunks of 128 output channels
RTR = 8                           # rows per matmul tile
RT = H // RTR                     # 7 row tiles per image
NTILE = RTR * W                   # 448 = matmul free dim (<=512, one PSUM bank)
NTOT = N_FULL * PIX               # 200704 elements per channel for BN stats
NPAIR = 14                        # DoubleRow matmuls per PSUM tile
HFX = PIX // 2                    # half-image columns (sign/DMA granularity)
WSLOT = 2 * 128                   # fp8 bytes per weight pair
PHA = 9 * WSLOT                   # phase-1 weight bytes (pairs 0-8)
PHB = 5 * WSLOT                   # phase-2 weight bytes (pairs 9-13)

BF16 = mybir.dt.bfloat16
F32 = mybir.dt.float32
FP8 = mybir.dt.float8e4
NP8 = ml_dtypes.float8_e4m3

_MAX_DRAIN_WAITS = 1  # walrus CTRL instructions accept a single sync wait


def _split_multi_waits(nc, max_waits=1):
    """This walrus build rejects instructions with more than one sem wait.
    Hoist excess waits onto same-engine NoOps inserted immediately before the
    offending instruction (the engine blocks at the NoOp instead — identical
    ordering semantics)."""
    ctr = 0
    for bbw in nc.main_func.blocks:
        out = []
        changed = False
        for inst in bbw.instructions:
            si = inst.sync_info
            w = list(si.on_wait or []) if si else []
            if len(w) > max_waits:
                changed = True
                excess = w[: len(w) - max_waits]
                for i in range(0, len(excess), max_waits):
                    nop = mybir.InstNoOp(name=f"WFIX-{ctr}", ins=[], outs=[])
                    ctr += 1
                    nop.engine = inst.engine
                    nop.sync_info = mybir.SyncInfo(
                        on_wait=excess[i : i + max_waits], on_update=[]
                    )
                    out.append(nop)
                inst.sync_info = mybir.SyncInfo(
                    on_wait=w[len(w) - max_waits :],
                    on_update=list(si.on_update or []),
                )
            out.append(inst)
        if changed:
            bbw.instructions = out
    return ctr


class _SplitDrainTileContext(tile.TileContext):
    """TileContext whose final drain splits its sem waits across multiple
    sync-engine instructions (this walrus build caps CTRL waits at 1)."""

    def _drain_and_barrier(self, tick_clock, wait_clock):
        drain_inst = self.nc.sync.drain()
        wait_clock.add_sem_waits(
            drain_inst.ins, ScopedClock({None: tick_clock.global_clock})
        )
        si = drain_inst.ins.sync_info
        w = list(si.on_wait or [])
        if len(w) > _MAX_DRAIN_WAITS:
            drain_inst.ins.sync_info = mybir.SyncInfo(
                on_wait=w[:_MAX_DRAIN_WAITS], on_update=list(si.on_update or [])
            )
            for i in range(_MAX_DRAIN_WAITS, len(w), _MAX_DRAIN_WAITS):
                nop = self.nc.sync.nop(nofuse=True)
                nop.ins.sync_info = mybir.SyncInfo(
                    on_wait=w[i : i + _MAX_DRAIN_WAITS], on_update=[]
                )
        self.nc.all_engine_barrier()
        assert self.sems is not None
        popped = self.nc._tile_sem_poison_stack.pop()
        assert popped is self._sem_poison
        self.nc.clear_and_free_semaphores(list(self.sems.allocated().values()))
        self.nc.all_engine_barrier()


def _pair_rhs(xa_v, xb_v, rt, pr):
    """rhs AP [128, 2, 8, 56] for DoubleRow pair pr of row-tile rt.

    xa_v: [128, 2, HP, WP] view of planes (q0, q1);
    xb_v: [128, 3, HP, WP] view of planes (q2, q2<<1, q2<<59)."""
    r0 = rt * RTR
    if pr < 9:                       # (tap pr, q0) + (tap pr, q1)
        dy, dx = divmod(pr, KW)
        return xa_v[:, 0:2, r0 + dy : r0 + dy + RTR, dx : dx + W]
    if pr < 12:                      # q2 taps (dy,0)+(dy,1) via shift-1 plane
        dy = pr - 9
        return xb_v[:, 0:2, r0 + dy : r0 + dy + RTR, 0:W]
    if pr == 12:                     # q2 taps (0,2)+(1,2) via shift-59 plane
        return xb_v[:, 1:3, r0 : r0 + RTR, 1 : 1 + W]
    # pr == 13: q2 tap (2,2), second half has zero weights
    return xb_v[:, 0:1, r0 + 2 : r0 + 2 + RTR, 2 : 2 + W].broadcast_to(
        [128, 2, RTR, W]
    )


def build_bass(n_cores=N_CORES, collective=True):
    """Build the per-core Bass module (SPMD: same program on every core)."""
    nc = bass.Bass(num_devices=n_cores)

    xq_d = nc.dram_tensor("xq", [IMGS, CIN, 5 * PADPIX], FP8,
                          kind="ExternalInput")
    wq_d = nc.dram_tensor("wq", [CIN, NCHUNK * NPAIR * WSLOT], FP8,
                          kind="ExternalInput")
    # pqrb[p, j-col chunks]: P | Qc | R | beta  (4 cols per chunk)
    pqrb_d = nc.dram_tensor("pqrb", [128, 4 * NCHUNK], F32,
                            kind="ExternalInput")
    out_d = nc.dram_tensor("out", [IMGS, NCHUNK, 128, PIX], FP8,
                           kind="ExternalOutput")

    with _SplitDrainTileContext(nc) as tc:
        with (
            tc.tile_pool(name="const", bufs=1) as constp,
            tc.tile_pool(name="xbuf", bufs=1) as xp,
            tc.tile_pool(name="zbuf", bufs=1) as zp,
            tc.tile_pool(name="stats", bufs=1) as sp,
            tc.tile_pool(name="sq", bufs=2) as sqp,
            tc.tile_pool(name="ost", bufs=6) as op_,
            tc.tile_pool(name="pz", bufs=8, space="PSUM") as pp,
            tc.tile_pool(name="dram", bufs=1, space="DRAM") as dp,
        ):
            # ---- weights (split per chunk/phase for early PE start) ----
            wa = [constp.tile([128, PHA], FP8, tag=f"wa{j}", name=f"wa{j}")
                  for j in range(NCHUNK)]
            wb = [constp.tile([128, PHB], FP8, tag=f"wb{j}", name=f"wb{j}")
                  for j in range(NCHUNK)]
            pqrb_sb = constp.tile([128, 4 * NCHUNK], F32, tag="pqrb")

            # ---- x plane stacks (per image, split planes 01 / 234) ----
            xa = [xp.tile([128, 2 * PADPIX], FP8, tag=f"xa{i}", name=f"xa{i}")
                  for i in range(IMGS)]
            xb = [xp.tile([128, 3 * PADPIX], FP8, tag=f"xb{i}", name=f"xb{i}")
                  for i in range(IMGS)]

            # DMA issue order = arrival order: w0a, img0 planes, w0b, then
            # the rest.  First matmul waits only on w0a + xa[0].
            nc.sync.dma_start(wa[0][:], wq_d[:, 0:PHA])
            xa0_v = xa[0][:].rearrange("p (t pix) -> p t pix", t=2)
            xq0_v = xq_d[0].rearrange("c (t pix) -> c t pix", t=5)[:, 0:2]
            RSPL = 30 * WP
            nc.sync.dma_start(xa0_v[:, :, 0:RSPL], xq0_v[:, :, 0:RSPL])
            nc.sync.dma_start(xa0_v[:, :, RSPL:], xq0_v[:, :, RSPL:])
            nc.sync.dma_start(xb[0][:], xq_d[0][:, 2 * PADPIX :])
            nc.sync.dma_start(wb[0][:], wq_d[:, PHA : PHA + PHB])
            nc.sync.dma_start(pqrb_sb[:], pqrb_d[:])
            for i in range(1, IMGS):
                nc.sync.dma_start(xa[i][:], xq_d[i][:, 0 : 2 * PADPIX])
                nc.sync.dma_start(xb[i][:], xq_d[i][:, 2 * PADPIX :])
            off = NPAIR * WSLOT
            nc.sync.dma_start(wa[1][:], wq_d[:, off : off + PHA])
            nc.sync.dma_start(wb[1][:], wq_d[:, off + PHA : off + PHA + PHB])

            # ---- PE warmup: dummy matmuls ramp the tensor-engine clock
            # to full speed while the x/w DMAs are in flight ----
            wu = constp.tile([128, 520], FP8, tag="warm")
            wscr = constp.tile([128, 1], F32, tag="wscr")
            nc.gpsimd.memset(wu[:], 0)
            pw = pp.tile([128, 512], F32, tag="pz", name="warm_ps")
            for i in range(9):
                nc.tensor.matmul(pw[0:1, :], wu[:, i : i + 1], wu[:, 8:520],
                                 start=True, stop=True)
            nc.vector.tensor_copy(wscr[0:1, 0:1], pw[0:1, 0:1])

            # ---- z buffers + stats ----
            z = [zp.tile([128, IMGS * PIX], F32, tag=f"z{j}", name=f"z{j}")
                 for j in range(NCHUNK)]
            ssum = sp.tile([128, 64], F32, tag="ssum")

            ssq = sp.tile([128, 64], F32, tag="ssq")

            P_ = pqrb_sb[:, 0:NCHUNK]
            Qc = pqrb_sb[:, NCHUNK : 2 * NCHUNK]
            R_ = pqrb_sb[:, 2 * NCHUNK : 3 * NCHUNK]
            beta = pqrb_sb[:, 3 * NCHUNK : 4 * NCHUNK]
            inv_n = 1.0 / NTOT
            npart = IMGS * RT

            def _consume(j, img, rt, pt):
                """Stats consumers for one finished PSUM tile: DVE z-copy
                with sum accumulation, ACT square with sumsq accumulation."""
                col = img * RT + rt
                zs = z[j][:, img * PIX + rt * NTILE
                          : img * PIX + (rt + 1) * NTILE]
                nc.vector.tensor_scalar(
                    out=zs, in0=pt[:], scalar1=0.0, scalar2=None,
                    op0=mybir.AluOpType.add, op1=mybir.AluOpType.add,
                    accum_out=ssum[:, j * npart + col : j * npart + col + 1],
                )
                sqt = sqp.tile([128, NTILE], F32, tag="sqt")
                nc.scalar.activation(
                    out=sqt[:], in_=pt[:],
                    func=mybir.ActivationFunctionType.Square,
                    accum_out=ssq[:, j * npart + col : j * npart + col + 1],
                )

            def _mm(pt, wv, xa_v, xb_v, rt, pr):
                nc.tensor.matmul(
                    pt[:], wv[:, pr] if pr < 9 else wv[:, pr - 9],
                    _pair_rhs(xa_v, xb_v, rt, pr),
                    start=(pr == 0), stop=(pr == 12),
                    perf_mode=mybir.MatmulPerfMode.DoubleRow,
                )

            TILE_ORDER = list(range(9)) + [9, 10, 11, 13, 12]

            def _conv_img(j, img):
                wa_v = wa[j][:].rearrange("p (pr k o) -> p pr k o", pr=9, k=2)
                wb_v = wb[j][:].rearrange("p (pr k o) -> p pr k o", pr=5, k=2)
                xa_v = xa[img][:].rearrange("p (t r c) -> p t r c",
                                            t=2, r=HP)
                xb_v = xb[img][:].rearrange("p (t r c) -> p t r c",
                                            t=3, r=HP)
                pts = [pp.tile([128, NTILE], F32, tag="pz",
                               name=f"pz{j}_{img}_{rt}")
                       for rt in range(RT)]
                if j == 0 and img == 0:
                    # Phased: pairs 0-8 (planes q0/q1, early DMA) across
                    # all row tiles first — a 63-matmul runway while
                    # xb[0] is still in flight.
                    for rt in range(RT):
                        for pr in range(9):
                            _mm(pts[rt], wa_v, xa_v, xb_v, rt, pr)
                    for rt in range(RT):
                        for pr in (9, 10, 11, 13, 12):
                            _mm(pts[rt], wb_v, xa_v, xb_v, rt, pr)
                        _consume(j, img, rt, pts[rt])
                else:
                    # Tile-major: each tile's 14 pairs are consecutive so
                    # group-closures are spaced a full tile apart and the
                    # per-tile consumers keep up.
                    for rt in range(RT):
                        for pr in TILE_ORDER:
                            _mm(pts[rt], wa_v if pr < 9 else wb_v,
                                xa_v, xb_v, rt, pr)
                        _consume(j, img, rt, pts[rt])

            def _stats_bn(j):
                # ---- chunk-j stats: [128,2] = (sum, sumsq) ----
                cc_sb = sp.tile([128, 2], F32, tag=f"ccsb{j}", name=f"ccsb{j}")
                nc.vector.reduce_sum(
                    out=cc_sb[:, 0:1], in_=ssum[:, j * npart : (j + 1) * npart],
                    axis=mybir.AxisListType.X,
                )
                nc.vector.reduce_sum(
                    out=cc_sb[:, 1:2], in_=ssq[:, j * npart : (j + 1) * npart],
                    axis=mybir.AxisListType.X,
                )
                st = sp.tile([128, 2], F32, tag=f"st{j}", name=f"st{j}")
                if collective and n_cores > 1:
                    cc_in = dp.tile([128, 2], F32, tag=f"ccin{j}",
                                    name=f"ccin{j}")
                    cc_out = dp.tile([128, 2], F32, tag=f"ccout{j}",
                                     name=f"ccout{j}")
                    nc.sync.dma_start(cc_in[:], cc_sb[:])
                    nc.gpsimd.collective_compute(
                        "AllReduce", mybir.AluOpType.add,
                        replica_groups=[list(range(n_cores))],
                        ins=[cc_in.opt()], outs=[cc_out.opt()],
                    )
                    nc.sync.dma_start(st[:], cc_out[:])
                else:
                    nc.gpsimd.tensor_copy(st[:], cc_sb[:])

                # ---- BN affine:  A = R*rsqrt(P*q - Qc*s^2 + eps) >= 0,
                #      B = beta - mu*A,  T = mu - beta/A  (tail chunk only).
                # Chunk 0's math runs on Pool (DVE is busy with tile
                # consumers of chunk 1); the tail chunk's runs on DVE.
                last = j == NCHUNK - 1
                eng = nc.vector if last else nc.gpsimd
                Pj, Qj, Rj, bj = (v[:, j : j + 1] for v in (P_, Qc, R_, beta))
                s0, s1 = st[:, 0:1], st[:, 1:2]
                mu = sp.tile([128, 1], F32, tag=f"mu{j}", name=f"mu{j}")
                u = sp.tile([128, 1], F32, tag=f"u{j}", name=f"u{j}")
                A = sp.tile([128, 1], F32, tag=f"A{j}", name=f"A{j}")
                B = sp.tile([128, 1], F32, tag=f"B{j}", name=f"B{j}")
                t1 = sp.tile([128, 1], F32, tag=f"t1{j}", name=f"t1{j}")

                eng.tensor_tensor(out=u[:], in0=s1, in1=Pj,
                                  op=mybir.AluOpType.mult)
                eng.tensor_tensor(out=t1[:], in0=s0, in1=s0,
                                  op=mybir.AluOpType.mult)
                eng.tensor_tensor(out=t1[:], in0=t1[:], in1=Qj,
                                  op=mybir.AluOpType.mult)
                eng.tensor_tensor(out=u[:], in0=u[:], in1=t1[:],
                                  op=mybir.AluOpType.subtract)
                eng.tensor_scalar(out=u[:], in0=u[:], scalar1=float(BN_EPS),
                                  scalar2=None, op0=mybir.AluOpType.add)
                nc.vector.reciprocal(u[:], u[:])
                nc.scalar.activation(out=u[:], in_=u[:],
                                     func=mybir.ActivationFunctionType.Sqrt)
                eng.tensor_tensor(out=A[:], in0=Rj, in1=u[:],
                                  op=mybir.AluOpType.mult)
                eng.tensor_scalar(out=mu[:], in0=s0, scalar1=inv_n,
                                  scalar2=None, op0=mybir.AluOpType.mult)
                eng.tensor_tensor(out=t1[:], in0=mu[:], in1=A[:],
                                  op=mybir.AluOpType.mult)
                eng.tensor_tensor(out=B[:], in0=bj, in1=t1[:],
                                  op=mybir.AluOpType.subtract)
                return A, B, mu, t1, bj

            def _act_sign(j, AB, img, h):
                """ACT Sign on half-image h of img -> staged -> DRAM."""
                A, B = AB
                o = op_.tile([128, HFX], FP8, tag="ostg",
                             name=f"ostg{j}_{img}_{h}")
                lo = img * PIX + h * HFX
                nc.scalar.activation(
                    out=o[:], in_=z[j][:, lo : lo + HFX],
                    func=mybir.ActivationFunctionType.Sign,
                    bias=B[:, 0:1], scale=A[:, 0:1],
                )
                nc.sync.dma_start(
                    out_d[img, j][:, h * HFX : (h + 1) * HFX], o[:])

            # ---- emission schedule: chunk 0's stats/BN/signs are emitted
            # inside chunk 1's conv stream so its ACT sign work interleaves
            # with chunk 1's Squares without head-of-line blocking. ----
            for img in range(IMGS):
                _conv_img(0, img)
            _conv_img(1, 0)
            A0, B0, _, _, _ = _stats_bn(0)
            sched0 = [[(0, 0), (0, 1), (1, 0)],       # after conv(1, img=1)
                      [(1, 1), (2, 0), (2, 1)],       # after conv(1, img=2)
                      [(3, 0), (3, 1)]]               # after conv(1, img=3)
            for img in range(1, IMGS):
                _conv_img(1, img)
                for (si, sh) in sched0[img - 1]:
                    _act_sign(0, (A0, B0), si, sh)

            # ---- tail chunk: ACT takes imgs 0-1, DVE takes imgs 2-3 as
            # single-pass compares (z >= T) -> {1,0}; the host maps
            # {1,0} -> {+1,-1} for exactly these slices. ----
            A1, B1, mu1, t1_, bj1 = _stats_bn(1)
            T = sp.tile([128, 1], F32, tag="T", name="T")
            nc.vector.reciprocal(t1_[:], A1[:])
            nc.vector.tensor_tensor(out=t1_[:], in0=bj1, in1=t1_[:],
                                    op=mybir.AluOpType.mult)
            nc.vector.tensor_tensor(out=T[:], in0=mu1[:], in1=t1_[:],
                                    op=mybir.AluOpType.subtract)

            def _dve_cmp(img, h):
                o = op_.tile([128, HFX], FP8, tag="ostg",
                             name=f"ostg1_{img}_{h}")
                lo = img * PIX + h * HFX
                nc.vector.tensor_tensor(
                    out=o[:], in0=z[1][:, lo : lo + HFX],
                    in1=T[:, 0:1].broadcast_to([128, HFX]),
                    op=mybir.AluOpType.is_ge,
                )
                nc.sync.dma_start(
                    out_d[img, 1][:, h * HFX : (h + 1) * HFX], o[:])

            # interleaved by expected completion so the in-order SP DMA
            # queue never head-of-line blocks on a not-yet-ready half
            for (use_act, img, h) in (
                (True, 0, 0), (False, 2, 0),
                (True, 0, 1), (False, 2, 1),
                (True, 1, 0), (False, 3, 0),
                (True, 1, 1), (False, 3, 1),
            ):
                if use_act:
                    _act_sign(1, (A1, B1), img, h)
                else:
                    _dve_cmp(img, h)

    _split_multi_waits(nc)
    return nc


def _prep_inputs(x, weight, gamma, beta):
    """Host-side prep: sign/alpha/gamma folding, padding, 3-term fp8 split."""
    x = np.ascontiguousarray(x, dtype=np.float32)
    weight = np.ascontiguousarray(weight, dtype=np.float32)
    gamma = np.asarray(gamma, np.float32)
    beta = np.asarray(beta, np.float32)

    alpha = np.abs(weight).mean(axis=(1, 2, 3)).astype(np.float32)      # [256]
    S = np.where(gamma >= 0, np.float32(1), np.float32(-1))
    sgn = np.where(weight >= 0, np.float32(1), np.float32(-1)) * S[:, None, None, None]

    # ---- fp8 weight pairs: wq[cin, j, pair, ktile, o] ----
    sgn_t = sgn.transpose(1, 2, 3, 0).reshape(CIN, KH * KW, NCHUNK, 128)
    wq = np.zeros((CIN, NCHUNK, NPAIR, 2, 128), np.float32)
    S1, S2 = 2.0 ** -4, 2.0 ** -6
    for k in range(9):                      # pairs 0-8: (q0, q1) of tap k
        wq[:, :, k, 0] = sgn_t[:, k]
        wq[:, :, k, 1] = sgn_t[:, k] * S1
    for dy in range(3):                     # pairs 9-11: q2 taps (dy,0)+(dy,1)
        wq[:, :, 9 + dy, 0] = sgn_t[:, dy * 3 + 0] * S2
        wq[:, :, 9 + dy, 1] = sgn_t[:, dy * 3 + 1] * S2
    wq[:, :, 12, 0] = sgn_t[:, 2] * S2      # pair 12: q2 taps (0,2)+(1,2)
    wq[:, :, 12, 1] = sgn_t[:, 5] * S2
    wq[:, :, 13, 0] = sgn_t[:, 8] * S2      # pair 13: q2 tap (2,2) + zeros
    wq = np.ascontiguousarray(
        wq.reshape(CIN, NCHUNK * NPAIR * WSLOT)
    ).astype(NP8)

    # pqrb[p, j]: P = a^2/N | Qc = a^2/N^2 | R = a*|g| | beta
    def chunked(v):
        return np.ascontiguousarray(v.reshape(NCHUNK, 128).T)  # [128, 2]
    a2 = alpha * alpha
    pqrb = np.concatenate(
        [chunked(a2 / NTOT), chunked(a2 / NTOT / NTOT),
         chunked(alpha * np.abs(gamma)), chunked(beta)], axis=1
    ).astype(np.float32)                                                # [128, 8]

    # ---- 3-term fp8 split of padded x, with shifted q2 planes ----
    xpad = np.zeros((N_FULL, CIN, HP, WP), np.float32)
    xpad[:, :, 1 : H + 1, 1 : W + 1] = x
    xpad = xpad.reshape(N_FULL, CIN, PADPIX)
    q0 = xpad.astype(NP8)
    r1 = xpad - q0.astype(np.float32)
    q1 = (r1 * 16.0).astype(NP8)
    r2 = r1 - q1.astype(np.float32) * (1.0 / 16.0)
    q2 = (r2 * 64.0).astype(NP8)
    q2p = np.zeros((N_FULL, CIN, PADPIX + 64), NP8)
    q2p[:, :, :PADPIX] = q2
    xq = np.stack(
        [q0, q1, q2, q2p[:, :, 1 : 1 + PADPIX], q2p[:, :, 59 : 59 + PADPIX]],
        axis=2,
    )                                                   # [N, CIN, 5, PADPIX]
    xq = np.ascontiguousarray(xq.reshape(N_FULL, CIN, 5 * PADPIX))

    in_maps = []
    for c in range(N_CORES):
        sl = slice(c * IMGS, (c + 1) * IMGS)
        in_maps.append({
            "xq": np.ascontiguousarray(xq[sl]),
            "wq": wq,
            "pqrb": pqrb,
        })
    return in_maps


def kernel(x, weight, gamma, beta):
    in_maps = _prep_inputs(x, weight, gamma, beta)
    nc = build_bass()
    res = run_bass_kernel_spmd(nc, in_maps, core_ids=list(range(N_CORES)))
    out = np.empty((N_FULL, COUT, H, W), np.float32)
    for c in range(N_CORES):
        o = res.results[c]["out"].astype(np.float32)   # [IMGS,2,128,PIX] fp8
        o[2:4, 1] = o[2:4, 1] * 2.0 - 1.0   # DVE compare slices: {1,0}->{+-1}
        out[c * IMGS : (c + 1) * IMGS] = o.reshape(IMGS, COUT, H, W)
    return out


# revision 20
# speedup vs baseline: 2.4148x; 1.0105x over previous
"""Trainium2 Bass kernel for ConvBnSign (binarized 3x3 conv + sync-BN + sign).

Math: y = conv2d(x, sign(w) * alpha)  with alpha = mean|w| per out-channel,
then train-mode BatchNorm over (N,H,W), then hard_sign.

Folds: alpha > 0 folds into the BN affine; S = sign(gamma) folds into the
binarized weights (z' = S*z), making the BN scale A = alpha*|gamma|*rsqrt(
alpha^2 var + eps) >= 0 so the final sign is also a per-channel threshold
compare  out = (z' >= T) ? +1 : -1,  T = mu' - beta/A  — which lets the
tail's sign pass split across ACT / DVE / Pool engines.

Precision: x is split on host into 3 fp8-e4m3 terms
  x ~ q0 + q1/16 + q2/64   (residual rms ~1.6e-5 relative),
with per-term scales folded into the fp8 weights (1, 2^-4, 2^-6 — all
normal in e4m3). Each conv tile is 14 DoubleRow fp8 matmuls (2 k-tiles
each, 0.5 cyc/row): the 27 (tap, term) k-tiles pair via the plane dim of
the SBUF layout, where two extra planes are host-shifted copies of q2
(shift +1 / +59) so cross-tap pairs land at the uniform plane stride.

Per-tile stats consumers both run on DVE (tensor_scalar z-copy+sum and
tensor_tensor_reduce square+sum), keeping ACT free for Sign and Pool free
for reduces/BN math — avoids ACT head-of-line blocking on PSUM drains.

Sharding: data-parallel, 4 images per core across 8 cores; BN stats are
per-channel partial sums [128,4] fp32 all-reduced across cores.
"""

import numpy as np
import ml_dtypes

import concourse.bass as bass
import concourse.mybir as mybir
import concourse.tile as tile
from concourse.vector_clock import ScopedClock
from concourse.bass_utils import run_bass_kernel_spmd

# ---- problem constants (hardcoded per contract) ----
N_CORES = 8
N_FULL = 32           # batch
CIN = 128             # input channels
COUT = 256            # output channels
H = W = 56
KH = KW = 3
BN_EPS = 1e-5

IMGS = N_FULL // N_CORES          # 4 images per core
WP = W + 2                        # 58 padded width
HP = H + 2
PADPIX = HP * WP                  # 3364
PIX = H * W                       # 3136
NCHUNK = COUT // 128              # 2 chunks of 128 output channels
RTR = 8                           # rows per matmul tile
RT = H // RTR                     # 7 row tiles per image
NTILE = RTR * W                   # 448 = matmul free dim (<=512, one PSUM bank)
NTOT = N_FULL * PIX               # 200704 elements per channel for BN stats
NPAIR = 14                        # DoubleRow matmuls per PSUM tile
HFX = PIX // 2                    # half-image columns (sign/DMA granularity)
WSLOT = 2 * 128                   # fp8 bytes per weight pair
PHA = 9 * WSLOT                   # phase-1 weight bytes (pairs 0-8)
PHB = 5 * WSLOT                   # phase-2 weight bytes (pairs 9-13)

BF16 = mybir.dt.bfloat16
F32 = mybir.dt.float32
FP8 = mybir.dt.float8e4
NP8 = ml_dtypes.float8_e4m3

_MAX_DRAIN_WAITS = 1  # walrus CTRL instructions accept a single sync wait


def _split_multi_waits(nc, max_waits=1):
    """This walrus build rejects instructions with more than one sem wait.
    Hoist excess waits onto same-engine NoOps inserted immediately before the
    offending instruction (the engine blocks at the NoOp instead — identical
    ordering semantics)."""
    ctr = 0
    for bbw in nc.main_func.blocks:
        out = []
        changed = False
        for inst in bbw.instructions:
            si = inst.sync_info
            w = list(si.on_wait or []) if si else []
            if len(w) > max_waits:
                changed = True
                excess = w[: len(w) - max_waits]
                for i in range(0, len(excess), max_waits):
                    nop = mybir.InstNoOp(name=f"WFIX-{ctr}", ins=[], outs=[])
                    ctr += 1
                    nop.engine = inst.engine
                    nop.sync_info = mybir.SyncInfo(
                        on_wait=excess[i : i + max_waits], on_update=[]
                    )
                    out.append(nop)
                inst.sync_info = mybir.SyncInfo(
                    on_wait=w[len(w) - max_waits :],
                    on_update=list(si.on_update or []),
                )
            out.append(inst)
        if changed:
            bbw.instructions = out
    return ctr


class _SplitDrainTileContext(tile.TileContext):
    """TileContext whose final drain splits its sem waits across multiple
    sync-engine instructions (this walrus build caps CTRL waits at 1)."""

    def _drain_and_barrier(self, tick_clock, wait_clock):
        drain_inst = self.nc.sync.drain()
        wait_clock.add_sem_waits(
            drain_inst.ins, ScopedClock({None: tick_clock.global_clock})
        )
        si = drain_inst.ins.sync_info
        w = list(si.on_wait or [])
        if len(w) > _MAX_DRAIN_WAITS:
            drain_inst.ins.sync_info = mybir.SyncInfo(
                on_wait=w[:_MAX_DRAIN_WAITS], on_update=list(si.on_update or [])
            )
            for i in range(_MAX_DRAIN_WAITS, len(w), _MAX_DRAIN_WAITS):
                nop = self.nc.sync.nop(nofuse=True)
                nop.ins.sync_info = mybir.SyncInfo(
                    on_wait=w[i : i + _MAX_DRAIN_WAITS], on_update=[]
                )
        self.nc.all_engine_barrier()
        assert self.sems is not None
        popped = self.nc._tile_sem_poison_stack.pop()
        assert popped is self._sem_poison
        self.nc.clear_and_free_semaphores(list(self.sems.allocated().values()))
        self.nc.all_engine_barrier()


def _pair_rhs(xa_v, xb_v, rt, pr):
    """rhs AP [128, 2, 8, 56] for DoubleRow pair pr of row-tile rt.

    xa_v: [128, 2, HP, WP] view of planes (q0, q1);
    xb_v: [128, 3, HP, WP] view of planes (q2, q2<<1, q2<<59)."""
    r0 = rt * RTR
    if pr < 9:                       # (tap pr, q0) + (tap pr, q1)
        dy, dx = divmod(pr, KW)
        return xa_v[:, 0:2, r0 + dy : r0 + dy + RTR, dx : dx + W]
    if pr < 12:                      # q2 taps (dy,0)+(dy,1) via shift-1 plane
        dy = pr - 9
        return xb_v[:, 0:2, r0 + dy : r0 + dy + RTR, 0:W]
    if pr == 12:                     # q2 taps (0,2)+(1,2) via shift-59 plane
        return xb_v[:, 1:3, r0 : r0 + RTR, 1 : 1 + W]
    # pr == 13: q2 tap (2,2), second half has zero weights
    return xb_v[:, 0:1, r0 + 2 : r0 + 2 + RTR, 2 : 2 + W].broadcast_to(
        [128, 2, RTR, W]
    )


def build_bass(n_cores=N_CORES, collective=True):
    """Build the per-core Bass module (SPMD: same program on every core)."""
    nc = bass.Bass(num_devices=n_cores)

    xq_d = nc.dram_tensor("xq", [IMGS, CIN, 5 * PADPIX], FP8,
                          kind="ExternalInput")
    wq_d = nc.dram_tensor("wq", [CIN, NCHUNK * NPAIR * WSLOT], FP8,
                          kind="ExternalInput")
    # pqrb[p, j-col chunks]: P | Qc | R | beta  (4 cols per chunk)
    pqrb_d = nc.dram_tensor("pqrb", [128, 4 * NCHUNK], F32,
                            kind="ExternalInput")
    out_d = nc.dram_tensor("out", [IMGS, NCHUNK, 128, PIX], FP8,
                           kind="ExternalOutput")

    with _SplitDrainTileContext(nc) as tc:
        with (
            tc.tile_pool(name="const", bufs=1) as constp,
            tc.tile_pool(name="xbuf", bufs=1) as xp,
            tc.tile_pool(name="zbuf", bufs=1) as zp,
            tc.tile_pool(name="stats", bufs=1) as sp,
            tc.tile_pool(name="sq", bufs=2) as sqp,
            tc.tile_pool(name="ost", bufs=6) as op_,
            tc.tile_pool(name="pz", bufs=8, space="PSUM") as pp,
            tc.tile_pool(name="dram", bufs=1, space="DRAM") as dp,
        ):
            # ---- weights (split per chunk/phase for early PE start) ----
            wa = [constp.tile([128, PHA], FP8, tag=f"wa{j}", name=f"wa{j}")
                  for j in range(NCHUNK)]
            wb = [constp.tile([128, PHB], FP8, tag=f"wb{j}", name=f"wb{j}")
                  for j in range(NCHUNK)]
            pqrb_sb = constp.tile([128, 4 * NCHUNK], F32, tag="pqrb")

            # ---- x plane stacks (per image, split planes 01 / 234) ----
            xa = [xp.tile([128, 2 * PADPIX], FP8, tag=f"xa{i}", name=f"xa{i}")
                  for i in range(IMGS)]
            xb = [xp.tile([128, 3 * PADPIX], FP8, tag=f"xb{i}", name=f"xb{i}")
                  for i in range(IMGS)]

            # DMA issue order = arrival order: w0a, img0 planes, w0b, then
            # the rest.  First matmul waits only on w0a + xa[0].
            nc.sync.dma_start(wa[0][:], wq_d[:, 0:PHA])
            xa0_v = xa[0][:].rearrange("p (t pix) -> p t pix", t=2)
            xq0_v = xq_d[0].rearrange("c (t pix) -> c t pix", t=5)[:, 0:2]
            RSPL = 30 * WP
            nc.sync.dma_start(xa0_v[:, :, 0:RSPL], xq0_v[:, :, 0:RSPL])
            nc.sync.dma_start(xa0_v[:, :, RSPL:], xq0_v[:, :, RSPL:])
            nc.sync.dma_start(xb[0][:], xq_d[0][:, 2 * PADPIX :])
            nc.sync.dma_start(wb[0][:], wq_d[:, PHA : PHA + PHB])
            nc.sync.dma_start(pqrb_sb[:], pqrb_d[:])
            for i in range(1, IMGS):
                nc.sync.dma_start(xa[i][:], xq_d[i][:, 0 : 2 * PADPIX])
                nc.sync.dma_start(xb[i][:], xq_d[i][:, 2 * PADPIX :])
            off = NPAIR * WSLOT
            nc.sync.dma_start(wa[1][:], wq_d[:, off : off + PHA])
            nc.sync.dma_start(wb[1][:], wq_d[:, off + PHA : off + PHA + PHB])

            # ---- PE warmup: dummy matmuls ramp the tensor-engine clock
            # to full speed while the x/w DMAs are in flight ----
            wu = constp.tile([128, 8], FP8, tag="warm")
            wscr = constp.tile([128, 1], F32, tag="wscr")
            nc.gpsimd.memset(wu[:], 0)
            pw = pp.tile([128, 512], F32, tag="pz", name="warm_ps")
            for i in range(7):
                nc.tensor.matmul(pw[0:1, :], wu[:, i : i + 1],
                                 wu[:, 0:1].broadcast_to([128, 512]),
                                 start=True, stop=True)
            nc.vector.tensor_copy(wscr[0:1, 0:1], pw[0:1, 0:1])

            # ---- z buffers + stats ----
            z = [zp.tile([128, IMGS * PIX], F32, tag=f"z{j}", name=f"z{j}")
                 for j in range(NCHUNK)]
            ssum = sp.tile([128, 64], F32, tag="ssum")

            ssq = sp.tile([128, 64], F32, tag="ssq")

            P_ = pqrb_sb[:, 0:NCHUNK]
            Qc = pqrb_sb[:, NCHUNK : 2 * NCHUNK]
            R_ = pqrb_sb[:, 2 * NCHUNK : 3 * NCHUNK]
            beta = pqrb_sb[:, 3 * NCHUNK : 4 * NCHUNK]
            inv_n = 1.0 / NTOT
            npart = IMGS * RT

            def _consume(j, img, rt, pt):
                """Stats consumers for one finished PSUM tile: DVE z-copy
                with sum accumulation, ACT square with sumsq accumulation."""
                col = img * RT + rt
                zs = z[j][:, img * PIX + rt * NTILE
                          : img * PIX + (rt + 1) * NTILE]
                nc.vector.tensor_scalar(
                    out=zs, in0=pt[:], scalar1=0.0, scalar2=None,
                    op0=mybir.AluOpType.add, op1=mybir.AluOpType.add,
                    accum_out=ssum[:, j * npart + col : j * npart + col + 1],
                )
                sqt = sqp.tile([128, NTILE], F32, tag="sqt")
                nc.scalar.activation(
                    out=sqt[:], in_=pt[:],
                    func=mybir.ActivationFunctionType.Square,
                    accum_out=ssq[:, j * npart + col : j * npart + col + 1],
                )

            def _mm(pt, wv, xa_v, xb_v, rt, pr):
                nc.tensor.matmul(
                    pt[:], wv[:, pr] if pr < 9 else wv[:, pr - 9],
                    _pair_rhs(xa_v, xb_v, rt, pr),
                    start=(pr == 0), stop=(pr == 12),
                    perf_mode=mybir.MatmulPerfMode.DoubleRow,
                )

            TILE_ORDER = list(range(9)) + [9, 10, 11, 13, 12]

            def _conv_img(j, img):
                wa_v = wa[j][:].rearrange("p (pr k o) -> p pr k o", pr=9, k=2)
                wb_v = wb[j][:].rearrange("p (pr k o) -> p pr k o", pr=5, k=2)
                xa_v = xa[img][:].rearrange("p (t r c) -> p t r c",
                                            t=2, r=HP)
                xb_v = xb[img][:].rearrange("p (t r c) -> p t r c",
                                            t=3, r=HP)
                pts = [pp.tile([128, NTILE], F32, tag="pz",
                               name=f"pz{j}_{img}_{rt}")
                       for rt in range(RT)]
                if j == 0 and img == 0:
                    # Phased: pairs 0-8 (planes q0/q1, early DMA) across
                    # all row tiles first — a 63-matmul runway while
                    # xb[0] is still in flight.
                    for rt in range(RT):
                        for pr in range(9):
                            _mm(pts[rt], wa_v, xa_v, xb_v, rt, pr)
                    for rt in range(RT):
                        for pr in (9, 10, 11, 13, 12):
                            _mm(pts[rt], wb_v, xa_v, xb_v, rt, pr)
                        _consume(j, img, rt, pts[rt])
                else:
                    # Tile-major: each tile's 14 pairs are consecutive so
                    # group-closures are spaced a full tile apart and the
                    # per-tile consumers keep up.
                    for rt in range(RT):
                        for pr in TILE_ORDER:
                            _mm(pts[rt], wa_v if pr < 9 else wb_v,
                                xa_v, xb_v, rt, pr)
                        _consume(j, img, rt, pts[rt])

            def _stats_bn(j):
                # ---- chunk-j stats: [128,2] = (sum, sumsq) ----
                cc_sb = sp.tile([128, 2], F32, tag=f"ccsb{j}", name=f"ccsb{j}")
                nc.vector.reduce_sum(
                    out=cc_sb[:, 0:1], in_=ssum[:, j * npart : (j + 1) * npart],
                    axis=mybir.AxisListType.X,
                )
                nc.vector.reduce_sum(
                    out=cc_sb[:, 1:2], in_=ssq[:, j * npart : (j + 1) * npart],
                    axis=mybir.AxisListType.X,
                )
                st = sp.tile([128, 2], F32, tag=f"st{j}", name=f"st{j}")
                if collective and n_cores > 1:
                    cc_in = dp.tile([128, 2], F32, tag=f"ccin{j}",
                                    name=f"ccin{j}")
                    cc_out = dp.tile([128, 2], F32, tag=f"ccout{j}",
                                     name=f"ccout{j}")
                    nc.sync.dma_start(cc_in[:], cc_sb[:])
                    nc.gpsimd.collective_compute(
                        "AllReduce", mybir.AluOpType.add,
                        replica_groups=[list(range(n_cores))],
                        ins=[cc_in.opt()], outs=[cc_out.opt()],
                    )
                    nc.sync.dma_start(st[:], cc_out[:])
                else:
                    st = cc_sb

                # ---- BN affine:  A = R*rsqrt(P*q - Qc*s^2 + eps) >= 0,
                #      B = beta - mu*A,  T = mu - beta/A  (tail chunk only).
                # Chunk 0's math runs on Pool (DVE is busy with tile
                # consumers of chunk 1); the tail chunk's runs on DVE.
                last = j == NCHUNK - 1
                eng = nc.vector if last else nc.gpsimd
                Pj, Qj, Rj, bj = (v[:, j : j + 1] for v in (P_, Qc, R_, beta))
                s0, s1 = st[:, 0:1], st[:, 1:2]
                mu = sp.tile([128, 1], F32, tag=f"mu{j}", name=f"mu{j}")
                u = sp.tile([128, 1], F32, tag=f"u{j}", name=f"u{j}")
                A = sp.tile([128, 1], F32, tag=f"A{j}", name=f"A{j}")
                B = sp.tile([128, 1], F32, tag=f"B{j}", name=f"B{j}")
                t1 = sp.tile([128, 1], F32, tag=f"t1{j}", name=f"t1{j}")

                if last:
                    # fused via scalar_tensor_tensor (DVE-only instruction)
                    nc.vector.scalar_tensor_tensor(
                        out=t1[:], in0=s0, scalar=Qj, in1=s0,
                        op0=mybir.AluOpType.mult, op1=mybir.AluOpType.mult)
                    nc.vector.scalar_tensor_tensor(
                        out=u[:], in0=s1, scalar=Pj, in1=t1[:],
                        op0=mybir.AluOpType.mult,
                        op1=mybir.AluOpType.subtract)
                else:
                    eng.tensor_tensor(out=u[:], in0=s1, in1=Pj,
                                      op=mybir.AluOpType.mult)
                    eng.tensor_tensor(out=t1[:], in0=s0, in1=s0,
                                      op=mybir.AluOpType.mult)
                    eng.tensor_tensor(out=t1[:], in0=t1[:], in1=Qj,
                                      op=mybir.AluOpType.mult)
                    eng.tensor_tensor(out=u[:], in0=u[:], in1=t1[:],
                                      op=mybir.AluOpType.subtract)
                eng.tensor_scalar(out=u[:], in0=u[:], scalar1=float(BN_EPS),
                                  scalar2=None, op0=mybir.AluOpType.add)
                nc.vector.reciprocal(u[:], u[:])
                nc.scalar.activation(out=u[:], in_=u[:],
                                     func=mybir.ActivationFunctionType.Sqrt)
                eng.tensor_tensor(out=A[:], in0=Rj, in1=u[:],
                                  op=mybir.AluOpType.mult)
                eng.tensor_scalar(out=mu[:], in0=s0, scalar1=inv_n,
                                  scalar2=None, op0=mybir.AluOpType.mult)
                eng.tensor_tensor(out=t1[:], in0=mu[:], in1=A[:],
                                  op=mybir.AluOpType.mult)
                eng.tensor_tensor(out=B[:], in0=bj, in1=t1[:],
                                  op=mybir.AluOpType.subtract)
                return A, B, mu, t1, bj

            def _act_sign(j, AB, img, h):
                """ACT Sign on half-image h of img -> staged -> DRAM."""
                A, B = AB
                o = op_.tile([128, HFX], FP8, tag="ostg",
                             name=f"ostg{j}_{img}_{h}")
                lo = img * PIX + h * HFX
                nc.scalar.activation(
                    out=o[:], in_=z[j][:, lo : lo + HFX],
                    func=mybir.ActivationFunctionType.Sign,
                    bias=B[:, 0:1], scale=A[:, 0:1],
                )
                nc.sync.dma_start(
                    out_d[img, j][:, h * HFX : (h + 1) * HFX], o[:])

            # ---- emission schedule: chunk 0's stats/BN/signs are emitted
            # inside chunk 1's conv stream so its ACT sign work interleaves
            # with chunk 1's Squares without head-of-line blocking. ----
            for img in range(IMGS):
                _conv_img(0, img)
            _conv_img(1, 0)
            A0, B0, _, _, _ = _stats_bn(0)
            sched0 = [[(0, 0), (0, 1), (1, 0)],       # after conv(1, img=1)
                      [(1, 1), (2, 0), (2, 1)],       # after conv(1, img=2)
                      [(3, 0), (3, 1)]]               # after conv(1, img=3)
            for img in range(1, IMGS):
                _conv_img(1, img)
                for (si, sh) in sched0[img - 1]:
                    _act_sign(0, (A0, B0), si, sh)

            # ---- tail chunk: ACT takes imgs 0-1, DVE takes imgs 2-3 as
            # single-pass compares (z >= T) -> {1,0}; the host maps
            # {1,0} -> {+1,-1} for exactly these slices. ----
            A1, B1, mu1, t1_, bj1 = _stats_bn(1)
            T = sp.tile([128, 1], F32, tag="T", name="T")
            nc.vector.reciprocal(t1_[:], A1[:])
            nc.vector.tensor_tensor(out=t1_[:], in0=bj1, in1=t1_[:],
                                    op=mybir.AluOpType.mult)
            nc.vector.tensor_tensor(out=T[:], in0=mu1[:], in1=t1_[:],
                                    op=mybir.AluOpType.subtract)

            def _dve_cmp(img, h):
                o = op_.tile([128, HFX], FP8, tag="ostg",
                             name=f"ostg1_{img}_{h}")
                lo = img * PIX + h * HFX
                nc.vector.tensor_tensor(
                    out=o[:], in0=z[1][:, lo : lo + HFX],
                    in1=T[:, 0:1].broadcast_to([128, HFX]),
                    op=mybir.AluOpType.is_ge,
                )
                nc.sync.dma_start(
                    out_d[img, 1][:, h * HFX : (h + 1) * HFX], o[:])

            # interleaved by expected completion so the in-order SP DMA
            # queue never head-of-line blocks on a not-yet-ready half
            for (use_act, img, h) in (
                (True, 0, 0), (False, 2, 0),
                (True, 0, 1), (False, 2, 1),
                (True, 1, 0), (False, 3, 0),
                (True, 1, 1), (False, 3, 1),
            ):
                if use_act:
                    _act_sign(1, (A1, B1), img, h)
                else:
                    _dve_cmp(img, h)

    _split_multi_waits(nc)
    return nc


def _prep_inputs(x, weight, gamma, beta):
    """Host-side prep: sign/alpha/gamma folding, padding, 3-term fp8 split."""
    x = np.ascontiguousarray(x, dtype=np.float32)
    weight = np.ascontiguousarray(weight, dtype=np.float32)
    gamma = np.asarray(gamma, np.float32)
    beta = np.asarray(beta, np.float32)

    alpha = np.abs(weight).mean(axis=(1, 2, 3)).astype(np.float32)      # [256]
    S = np.where(gamma >= 0, np.float32(1), np.float32(-1))
    sgn = np.where(weight >= 0, np.float32(1), np.float32(-1)) * S[:, None, None, None]

    # ---- fp8 weight pairs: wq[cin, j, pair, ktile, o] ----
    sgn_t = sgn.transpose(1, 2, 3, 0).reshape(CIN, KH * KW, NCHUNK, 128)
    wq = np.zeros((CIN, NCHUNK, NPAIR, 2, 128), np.float32)
    S1, S2 = 2.0 ** -4, 2.0 ** -6
    for k in range(9):                      # pairs 0-8: (q0, q1) of tap k
        wq[:, :, k, 0] = sgn_t[:, k]
        wq[:, :, k, 1] = sgn_t[:, k] * S1
    for dy in range(3):                     # pairs 9-11: q2 taps (dy,0)+(dy,1)
        wq[:, :, 9 + dy, 0] = sgn_t[:, dy * 3 + 0] * S2
        wq[:, :, 9 + dy, 1] = sgn_t[:, dy * 3 + 1] * S2
    wq[:, :, 12, 0] = sgn_t[:, 2] * S2      # pair 12: q2 taps (0,2)+(1,2)
    wq[:, :, 12, 1] = sgn_t[:, 5] * S2
    wq[:, :, 13, 0] = sgn_t[:, 8] * S2      # pair 13: q2 tap (2,2) + zeros
    wq = np.ascontiguousarray(
        wq.reshape(CIN, NCHUNK * NPAIR * WSLOT)
    ).astype(NP8)

    # pqrb[p, j]: P = a^2/N | Qc = a^2/N^2 | R = a*|g| | beta
    def chunked(v):
        return np.ascontiguousarray(v.reshape(NCHUNK, 128).T)  # [128, 2]
    a2 = alpha * alpha
    pqrb = np.concatenate(
        [chunked(a2 / NTOT), chunked(a2 / NTOT / NTOT),
         chunked(alpha * np.abs(gamma)), chunked(beta)], axis=1
    ).astype(np.float32)                                                # [128, 8]

    # ---- 3-term fp8 split of padded x, with shifted q2 planes ----
    xpad = np.zeros((N_FULL, CIN, HP, WP), np.float32)
    xpad[:, :, 1 : H + 1, 1 : W + 1] = x
    xpad = xpad.reshape(N_FULL, CIN, PADPIX)
    q0 = xpad.astype(NP8)
    r1 = xpad - q0.astype(np.float32)
    q1 = (r1 * 16.0).astype(NP8)
    r2 = r1 - q1.astype(np.float32) * (1.0 / 16.0)
    q2 = (r2 * 64.0).astype(NP8)
    q2p = np.zeros((N_FULL, CIN, PADPIX + 64), NP8)
    q2p[:, :, :PADPIX] = q2
    xq = np.stack(
        [q0, q1, q2, q2p[:, :, 1 : 1 + PADPIX], q2p[:, :, 59 : 59 + PADPIX]],
        axis=2,
    )                                                   # [N, CIN, 5, PADPIX]
    xq = np.ascontiguousarray(xq.reshape(N_FULL, CIN, 5 * PADPIX))

    in_maps = []
    for c in range(N_CORES):
        sl = slice(c * IMGS, (c + 1) * IMGS)
        in_maps.append({
            "xq": np.ascontiguousarray(xq[sl]),
            "wq": wq,
            "pqrb": pqrb,
        })
    return in_maps


def kernel(x, weight, gamma, beta):
    in_maps = _prep_inputs(x, weight, gamma, beta)
    nc = build_bass()
    res = run_bass_kernel_spmd(nc, in_maps, core_ids=list(range(N_CORES)))
    out = np.empty((N_FULL, COUT, H, W), np.float32)
    for c in range(N_CORES):
        o = res.results[c]["out"].astype(np.float32)   # [IMGS,2,128,PIX] fp8
        o[2:4, 1] = o[2:4, 1] * 2.0 - 1.0   # DVE compare slices: {1,0}->{+-1}
        out[c * IMGS : (c + 1) * IMGS] = o.reshape(IMGS, COUT, H, W)
    return out
